# revision 3
# baseline (speedup 1.0000x reference)
"""Trainium2 Bass kernel for a batched HGNN layer.

Per batch b (N=4096 nodes, E=2048 hyperedges, C=128 channels):
    De = sum_n H[n,e] + eps                 (hyperedge degrees)
    Dv = sum_e H[n,e] + eps                 (node degrees)
    s  = 1/sqrt(Dv)
    out = ((H @ ((H^T @ (x * s)) / De)) * s) @ W^T + b

Sharding: batch dim B=8, one batch per NeuronCore (data parallel, no
cross-core communication). Inside a core:

  pass 1 (streams H once from HBM, per-chunk 1 MiB loads on the sync
          HWDGE ring, software-pipelined):
    - fp32->bf16 casts on ACT/DVE with accum_out giving Dv row-sums
    - out2T[c,e] = (x*s)^T @ H accumulated in PSUM (PE, bf16)
    - H^T built with PLAIN matmuls against an extended identity
      [I | 1]: each 130-col matmul yields the transposed 128x128 tile
      plus a De (hyperedge-degree) partial in column 128 -- no
      separate De matmuls, no accum_out on the staging copies.
    - stg PSUM (fp32) copied to a bf16 SBUF H^T cache laid out
      130 cols per n-chunk so one strided copy moves tile+De.
  interlude:
    - RecDe = 1/(sum De partials + eps) via one strided DVE reduce
    - out3[e,c] = transpose(out2T) * (1/De)
  pass 2 (H^T streamed from SBUF, no HBM traffic; 5 groups of column
          blocks so each group's epilogue overlaps the next group's
          matmul stream):
    - out4T[c,n] = out3^T @ H^T  (PE, bf16)
    - out[n,co] = (out4T_tile^T @ W^T)*s + b  (PE bf16, DVE epilogue)

HBM traffic per core = 32 MB (H) + 2 MB (x) + 2 MB (out) ~= 36 MB,
i.e. the memory roofline for this problem.
"""
import os
import sys

import numpy as np

for _p in ("/opt/trn_rl_repo", "/root/.axon_site/_ro/trn_rl_repo"):
    if os.path.isdir(_p) and _p not in sys.path:
        sys.path.append(_p)

B, N, E, C = 8, 4096, 2048, 128
SC = 4                      # subchunks (128 rows) per superchunk
KHT = 130                   # H^T cache block: 128 cols + De partial + pad
NSUPER = N // (128 * SC)    # 8 superchunks in pass 1
NCHUNKS = N // 128          # 32 row chunks
ETILES = E // 128           # 16 hyperedge tiles
PRE = 4                     # h32 chunk prefetch depth (1 MiB each)
EPS = 1e-6

_CACHE = {}


def _build_nc():
    from contextlib import ExitStack

    import concourse.tile as tile
    from concourse import bacc, mybir

    F32 = mybir.dt.float32
    BF16 = mybir.dt.bfloat16
    X = mybir.AxisListType.X
    XY = mybir.AxisListType.XY
    COPY = mybir.ActivationFunctionType.Copy
    ADD = mybir.AluOpType.add

    nc = bacc.Bacc("TRN2", target_bir_lowering=False, debug=False)

    H_d = nc.dram_tensor("H", [N, E], F32, kind="ExternalInput")
    x_d = nc.dram_tensor("x", [N, C], F32, kind="ExternalInput")
    W_d = nc.dram_tensor("Wt", [C, C], F32, kind="ExternalInput")
    b_d = nc.dram_tensor("b", [1, C], F32, kind="ExternalInput")
    out_d = nc.dram_tensor("out", [N, C], F32, kind="ExternalOutput")

    H_ap, x_ap, out_ap = H_d.ap(), x_d.ap(), out_d.ap()

    with tile.TileContext(nc) as tc:
        with ExitStack() as ctx:
            const = ctx.enter_context(tc.tile_pool(name="const", bufs=1))
            h32p = ctx.enter_context(tc.tile_pool(name="h32", bufs=PRE))
            h16p = ctx.enter_context(tc.tile_pool(name="h16", bufs=6))
            xpool = ctx.enter_context(tc.tile_pool(name="xp", bufs=2))
            spool = ctx.enter_context(tc.tile_pool(name="sp", bufs=2))
            opool = ctx.enter_context(tc.tile_pool(name="op", bufs=2))
            psT_cm = tc.tile_pool(name="psT", bufs=2, space="PSUM")
            psT = psT_cm.__enter__()
            psA_cm = tc.tile_pool(name="psA", bufs=1, space="PSUM")
            psA = psA_cm.__enter__()

            # --- constants -------------------------------------------------
            # Extended identity [I | 1 1]: a PLAIN matmul h16^T @ ident_ext
            # writes the transposed tile in cols 0..127 and the partition
            # sum (De partial) in col 128 (129 is a pad, also a sum).
            ident16 = const.tile([128, KHT], BF16)
            nc.vector.memset(ident16[:], 1.0)
            nc.gpsimd.affine_select(
                ident16[:, 0:128], ident16[:, 0:128], pattern=[[-1, 128]],
                base=0, channel_multiplier=1,
                compare_op=mybir.AluOpType.is_equal, fill=0.0,
            )
            ones1 = const.tile([1, 128], F32)
            nc.vector.memset(ones1[:], 1.0)

            # --- persistent state ------------------------------------------
            # H^T cache: per (j, chunk) block of 130 bf16 cols
            # (128 transposed cols + De partial + pad), 130 KB/partition.
            HT = const.tile([128, ETILES * NCHUNKS * KHT], BF16)
            out3 = const.tile([128, ETILES * 128], BF16)  # (H^T xs)/De, [e, c]
            Isd = const.tile([128, NCHUNKS], F32)        # 1/sqrt(Dv)
            DvRaw = const.tile([128, NCHUNKS], F32)
            RecDe = const.tile([128, ETILES], F32)
            wt16 = const.tile([128, 128], BF16)          # W^T: [c_in, c_out]
            b_bcast = const.tile([128, 128], F32)        # b replicated per row
            b_sb = const.tile([1, 128], F32)

            out2T_ps = psA.tile([128, E], F32)           # 4 PSUM banks

            HTr = HT[:].rearrange("p (j c k) -> p j c k", j=ETILES, c=NCHUNKS)

            h32_tiles = {}

            def load_chunk(ci):
                """DMA one 128-row chunk of H (1 MiB fp32) on the sync ring.

                The sync engine is otherwise idle so triggers issue
                immediately; per-chunk granularity lets the first cast
                start ~3 us after kernel start and keeps buffer recycling
                fine-grained so the 16 HW DMA engines stay dense.
                """
                h32 = h32p.tile([128, E], F32, tag="h32", name=f"h32_{ci}")
                nc.sync.dma_start(h32[:], H_ap[ci * 128:(ci + 1) * 128, :])
                h32_tiles[ci] = h32

            def compute(i):
                x_t = xpool.tile([128, SC, C], F32, tag="x")
                nc.gpsimd.dma_start(
                    x_t[:],
                    x_ap[i * SC * 128:(i + 1) * SC * 128, :].rearrange(
                        "(t p) c -> p t c", p=128
                    ),
                )
                # fp32->bf16 casts with Dv row-sums fused via accum_out,
                # alternating ACT/DVE. Each cast frees an h32 buffer, so
                # the prefetch of chunk ci+PRE is emitted right after it.
                h16s = []
                for t in range(SC):
                    ci = i * SC + t
                    h16 = h16p.tile([128, E], BF16, tag="h16",
                                    name=f"h16_{i}_{t}")
                    h32 = h32_tiles.pop(ci)
                    if t % 2 == 0:
                        nc.scalar.activation(
                            h16[:], h32[:], COPY,
                            accum_out=DvRaw[:, ci:ci + 1],
                        )
                    else:
                        nc.vector.tensor_scalar(
                            h16[:], h32[:], 0.0, None, ADD, ADD,
                            accum_out=DvRaw[:, ci:ci + 1],
                        )
                    h16s.append(h16)
                    if ci + PRE < NCHUNKS:
                        load_chunk(ci + PRE)

                rec = spool.tile([128, SC], F32, tag="rec")
                nc.vector.tensor_scalar_add(
                    rec[:], DvRaw[:, i * SC:(i + 1) * SC], EPS
                )
                nc.vector.reciprocal(rec[:], rec[:])
                nc.scalar.sqrt(Isd[:, i * SC:(i + 1) * SC], rec[:])

                xs16 = xpool.tile([128, SC, C], BF16, tag="xs")
                for t in range(SC):
                    ci = i * SC + t
                    if t % 2 == 0:
                        nc.scalar.mul(
                            xs16[:, t, :], x_t[:, t, :], Isd[:, ci:ci + 1]
                        )
                    else:
                        nc.vector.tensor_scalar_mul(
                            xs16[:, t, :], x_t[:, t, :], Isd[:, ci:ci + 1]
                        )

                # H^T tiles + De partials via plain matmuls vs [I | 1].
                # stg slot padded to 256 f32/row so each matmul dest stays
                # inside a PSUM bank. Emitted BEFORE the out2T matmuls:
                # transposes only need the casts, not the xs chain.
                for j in range(ETILES):
                    stg = psT.tile([128, SC, 256], F32, tag="stg")
                    for t in range(SC):
                        nc.tensor.matmul(
                            stg[:, t, 0:KHT],
                            h16s[t][:, j * 128:(j + 1) * 128],
                            ident16[:],
                            start=True, stop=True,
                        )
                    dest = HTr[:, j, i * SC:(i + 1) * SC, :]
                    # DVE copies are cheaper; ACT carries the 2 casts, so
                    # give DVE 9 of 16 and ACT 7.
                    if j in (0, 2, 4, 6, 8, 10, 12):
                        nc.scalar.copy(dest, stg[:, :, 0:KHT])
                    else:
                        nc.vector.tensor_copy(dest, stg[:, :, 0:KHT])

                for t in range(SC):
                    for s in range(4):
                        nc.tensor.matmul(
                            out2T_ps[:, s * 512:(s + 1) * 512],
                            xs16[:, t, :],
                            h16s[t][:, s * 512:(s + 1) * 512],
                            start=(i == 0 and t == 0),
                            stop=(i == NSUPER - 1 and t == SC - 1),
                        )

            for ci in range(PRE):
                load_chunk(ci)

            compute(0)

            # W / b prep AFTER superchunk 0 so nothing on the H critical
            # path (PE first transpose, ACT first cast) queues behind the
            # W DMA -> wt16 cast chain at startup.
            wt32 = spool.tile([128, 128], F32, tag="wt32")
            nc.gpsimd.dma_start(wt32[:], W_d.ap())
            nc.vector.tensor_copy(wt16[:], wt32[:])

            nc.gpsimd.dma_start(b_sb[:], b_d.ap())
            bb_ps = psT.tile([128, SC, 256], F32, tag="stg")
            nc.tensor.matmul(bb_ps[:, 0, 0:128], ones1[:], b_sb[:],
                             start=True, stop=True)
            nc.scalar.copy(b_bcast[:], bb_ps[:, 0, 0:128])

            for i in range(1, NSUPER):
                compute(i)

            # --- interlude: De totals, copy out2 out of PSUM ---------------
            # De partials live at col 128 of each 130-col H^T block.
            nc.vector.reduce_sum(RecDe[:], HTr[:, :, :, 128:129], axis=XY)
            nc.vector.tensor_scalar_add(RecDe[:], RecDe[:], EPS)
            nc.vector.reciprocal(RecDe[:], RecDe[:])

            # out2T lands (bf16) in out3's buffer; each e-tile is then
            # transposed out and the scaled result overwrites it in place.
            nc.scalar.copy(out3[:, 0:1024], out2T_ps[:, 0:1024])
            nc.vector.tensor_copy(out3[:, 1024:2048], out2T_ps[:, 1024:2048])

            psA_cm.__exit__(None, None, None)

            # --- pass 2: 5 groups of column blocks (2+2+2+1+1) --------------
            # Group 0's matmul stream is interleaved with the out3 build
            # (transpose + 1/De scale per e-tile); each group's epilogue is
            # emitted after the NEXT group's stream so PE stays dense, and
            # the last group is a single block to minimize the exposed tail.
            GROUPS = [[0, 1], [2, 3], [4, 5], [6], [7]]
            psB_cm = tc.tile_pool(name="psB", bufs=4, space="PSUM")
            psB = psB_cm.__enter__()

            o4 = {}
            for grp in GROUPS:
                for blk in grp:
                    o4[blk] = psB.tile([128, 512], F32, tag="o4",
                                       name=f"o4_{blk}")

            def jstream(g):
                for j in range(ETILES):
                    for blk in GROUPS[g]:
                        nc.tensor.matmul(
                            o4[blk][:],
                            out3[:, j * 128:(j + 1) * 128],
                            HTr[:, j, blk * 4:(blk + 1) * 4, 0:128],
                            start=(j == 0), stop=(j == ETILES - 1),
                        )

            # group 0 + out3 build, interleaved per e-tile
            for j in range(ETILES):
                t2 = psT.tile([128, SC, 256], F32, tag="stg",
                              name=f"t2_{j}")
                nc.tensor.matmul(
                    t2[:, 0, 0:128], out3[:, j * 128:(j + 1) * 128],
                    ident16[:, 0:128], start=True, stop=True,
                )
                if j % 2 == 0:
                    nc.scalar.mul(
                        out3[:, j * 128:(j + 1) * 128], t2[:, 0, 0:128],
                        RecDe[:, j:j + 1]
                    )
                else:
                    nc.vector.tensor_scalar_mul(
                        out3[:, j * 128:(j + 1) * 128], t2[:, 0, 0:128],
                        RecDe[:, j:j + 1]
                    )
                for blk in GROUPS[0]:
                    nc.tensor.matmul(
                        o4[blk][:],
                        out3[:, j * 128:(j + 1) * 128],
                        HTr[:, j, blk * 4:(blk + 1) * 4, 0:128],
                        start=(j == 0), stop=(j == ETILES - 1),
                    )

            def epilogue(g):
                for blk in GROUPS[g]:
                    # all o4sb copies on ACT: keeps them out of the DVE
                    # queue so lp matmuls never wait behind queued stt's
                    o4sb = opool.tile([128, 512], BF16, tag="o4sb")
                    nc.scalar.copy(o4sb[:], o4[blk][:])
                    # two half-stores per block so the final DMA starts as
                    # soon as its half of the results is ready
                    for h in range(2):
                        obig = opool.tile([128, 2, C], F32, tag="obig",
                                          name=f"obig{blk}_{h}")
                        for u in range(2):
                            idx = blk * 4 + h * 2 + u
                            lp = psT.tile([128, SC, 256], F32, tag="stg",
                                          name=f"lp_{blk}_{h}_{u}")
                            nc.tensor.matmul(
                                lp[:, 0, 0:128],
                                o4sb[:, (h * 2 + u) * 128:
                                     (h * 2 + u + 1) * 128],
                                wt16[:], start=True, stop=True,
                            )
                            nc.vector.scalar_tensor_tensor(
                                obig[:, u, :], lp[:, 0, 0:128],
                                Isd[:, idx:idx + 1], b_bcast[:],
                                mybir.AluOpType.mult, mybir.AluOpType.add,
                            )
                        r0 = blk * 512 + h * 256
                        nc.gpsimd.dma_start(
                            out_ap[r0:r0 + 256, :].rearrange(
                                "(t p) c -> p t c", p=128
                            ),
                            obig[:],
                        )

            jstream(1)
            epilogue(0)
            jstream(2)
            epilogue(1)
            jstream(3)
            epilogue(2)
            jstream(4)
            epilogue(3)
            epilogue(4)

            psB_cm.__exit__(None, None, None)
            psT_cm.__exit__(None, None, None)

    nc.compile()
    return nc


def _get_nc():
    if "nc" not in _CACHE:
        _CACHE["nc"] = _build_nc()
    return _CACHE["nc"]


def kernel(x, H, W, b):
    from concourse.bass_utils import run_bass_kernel_spmd

    nc = _get_nc()
    x = np.ascontiguousarray(x, dtype=np.float32)
    H = np.ascontiguousarray(H, dtype=np.float32)
    W = np.ascontiguousarray(W, dtype=np.float32)
    b2 = np.ascontiguousarray(b, dtype=np.float32).reshape(1, C)
    Wt = np.ascontiguousarray(W.T)
    in_maps = [
        {"x": x[c], "H": H[c], "Wt": Wt, "b": b2} for c in range(B)
    ]
    res = run_bass_kernel_spmd(nc, in_maps, core_ids=list(range(B)))
    return np.stack([res.results[c]["out"] for c in range(B)], axis=0)


# revision 7
# speedup vs baseline: 1.0819x; 1.0819x over previous
"""Trainium2 Bass kernel for a batched HGNN layer.

Per batch b (N=4096 nodes, E=2048 hyperedges, C=128 channels):
    De = sum_n H[n,e] + eps                 (hyperedge degrees)
    Dv = sum_e H[n,e] + eps                 (node degrees)
    s  = 1/sqrt(Dv)
    out = ((H @ ((H^T @ (x * s)) / De)) * s) @ W^T + b

Sharding: batch dim B=8, one batch per NeuronCore (data parallel, no
cross-core communication). Inside a core:

  pass 1 (streams H once from HBM, per-chunk 1 MiB loads on the sync
          HWDGE ring, software-pipelined):
    - fp32->bf16 casts on ACT/DVE with accum_out giving Dv row-sums
    - H^T built with PE transposes (bf16, staged via PSUM); the
      PSUM->SBUF staging copies carry accum_out, yielding the De
      (hyperedge-degree) partials for free -- no De matmuls on PE.
      (PE, not ACT/DVE, must stay under the DMA floor per superchunk;
      the ~35% accum tax on copies is the cheaper place to pay.)
    - out2T[c,e] = (x*s)^T @ H accumulated in PSUM (PE, bf16),
      emitted AFTER the transposes so PE never waits on the
      cast->Dv->rsqrt->xs chain.
  interlude:
    - RecDe = 1/(sum De partials + eps)
    - out3[e,c] = transpose(out2T) * (1/De)
  pass 2 (H^T streamed from SBUF, no HBM traffic; 5 groups of column
          blocks so each group's epilogue overlaps the next group's
          matmul stream):
    - out4T[c,n] = out3^T @ H^T  (PE, bf16)
    - out[n,co] = (out4T_tile^T @ W^T)*s + b  (PE bf16, DVE epilogue),
      stores split in 256-row halves so the tail DMA starts early.

HBM traffic per core = 32 MB (H) + 2 MB (x) + 2 MB (out) ~= 36 MB,
i.e. the memory roofline for this problem.
"""
import os
import sys

import numpy as np

for _p in ("/opt/trn_rl_repo", "/root/.axon_site/_ro/trn_rl_repo"):
    if os.path.isdir(_p) and _p not in sys.path:
        sys.path.append(_p)

B, N, E, C = 8, 4096, 2048, 128
SC = 4                      # subchunks (128 rows) per superchunk
NSUPER = N // (128 * SC)    # 8 superchunks in pass 1
NCHUNKS = N // 128          # 32 row chunks
ETILES = E // 128           # 16 hyperedge tiles
PRE = 4                     # h32 chunk prefetch depth (1 MiB each)
EPS = 1e-6

_CACHE = {}


def _build_nc():
    from contextlib import ExitStack

    import concourse.tile as tile
    from concourse import bacc, mybir

    F32 = mybir.dt.float32
    BF16 = mybir.dt.bfloat16
    X = mybir.AxisListType.X
    COPY = mybir.ActivationFunctionType.Copy
    ADD = mybir.AluOpType.add

    nc = bacc.Bacc("TRN2", target_bir_lowering=False, debug=False)

    H_d = nc.dram_tensor("H", [N, E], F32, kind="ExternalInput")
    x_d = nc.dram_tensor("x", [N, C], F32, kind="ExternalInput")
    W_d = nc.dram_tensor("Wt", [C, C], F32, kind="ExternalInput")
    b_d = nc.dram_tensor("b", [1, C], F32, kind="ExternalInput")
    out_d = nc.dram_tensor("out", [N, C], F32, kind="ExternalOutput")

    H_ap, x_ap, out_ap = H_d.ap(), x_d.ap(), out_d.ap()

    with tile.TileContext(nc) as tc:
        with ExitStack() as ctx:
            const = ctx.enter_context(tc.tile_pool(name="const", bufs=1))
            h32p = ctx.enter_context(tc.tile_pool(name="h32", bufs=PRE))
            h16p = ctx.enter_context(tc.tile_pool(name="h16", bufs=6))
            xpool = ctx.enter_context(tc.tile_pool(name="xp", bufs=2))
            spool = ctx.enter_context(tc.tile_pool(name="sp", bufs=2))
            opool = ctx.enter_context(tc.tile_pool(name="op", bufs=2))
            psT_cm = tc.tile_pool(name="psT", bufs=4, space="PSUM")
            psT = psT_cm.__enter__()
            psA_cm = tc.tile_pool(name="psA", bufs=1, space="PSUM")
            psA = psA_cm.__enter__()

            # --- constants -------------------------------------------------
            ident16 = const.tile([128, 128], BF16)
            nc.vector.memset(ident16[:], 1.0)
            nc.gpsimd.affine_select(
                ident16[:], ident16[:], pattern=[[-1, 128]], base=0,
                channel_multiplier=1, compare_op=mybir.AluOpType.is_equal,
                fill=0.0,
            )
            ones1 = const.tile([1, 128], F32)
            nc.vector.memset(ones1[:], 1.0)

            # --- persistent state ------------------------------------------
            HT = const.tile([128, ETILES * N], BF16)     # H^T cache, 128 KB/part
            out3 = const.tile([128, ETILES * 128], BF16)  # (H^T xs)/De, [e, c]
            Isd = const.tile([128, NCHUNKS], F32)        # 1/sqrt(Dv)
            DvRaw = const.tile([128, NCHUNKS], F32)
            DeP2 = const.tile([128, ETILES * NSUPER], F32)  # De partials
            RecDe = const.tile([128, ETILES], F32)
            wt16 = const.tile([128, 128], BF16)          # W^T: [c_in, c_out]
            b_bcast = const.tile([128, 128], F32)        # b replicated per row
            b_sb = const.tile([1, 128], F32)

            out2T_ps = psA.tile([128, E], F32)           # 4 PSUM banks

            HT3 = HT[:].rearrange("p (j n) -> p j n", j=ETILES)
            DeP3 = DeP2[:].rearrange("p (j i) -> p j i", j=ETILES)

            h32_tiles = {}

            def load_chunk(ci):
                """DMA one 128-row chunk of H (1 MiB fp32) on the sync ring.

                The sync engine is otherwise idle so triggers issue
                immediately; per-chunk granularity lets the first cast
                start ~3 us after kernel start and keeps buffer recycling
                fine-grained so the 16 HW DMA engines stay dense.
                """
                h32 = h32p.tile([128, E], F32, tag="h32", name=f"h32_{ci}")
                nc.sync.dma_start(h32[:], H_ap[ci * 128:(ci + 1) * 128, :])
                h32_tiles[ci] = h32

            def compute(i):
                x_t = xpool.tile([128, SC, C], F32, tag="x")
                nc.gpsimd.dma_start(
                    x_t[:],
                    x_ap[i * SC * 128:(i + 1) * SC * 128, :].rearrange(
                        "(t p) c -> p t c", p=128
                    ),
                )
                # fp32->bf16 casts with Dv row-sums fused via accum_out,
                # alternating ACT/DVE. Each cast frees an h32 buffer, so
                # the prefetch of chunk ci+PRE is emitted right after it.
                h16s = []
                for t in range(SC):
                    ci = i * SC + t
                    h16 = h16p.tile([128, E], BF16, tag="h16",
                                    name=f"h16_{i}_{t}")
                    h32 = h32_tiles.pop(ci)
                    if t % 2 == 0:
                        nc.scalar.activation(
                            h16[:], h32[:], COPY,
                            accum_out=DvRaw[:, ci:ci + 1],
                        )
                    else:
                        nc.vector.tensor_scalar(
                            h16[:], h32[:], 0.0, None, ADD, ADD,
                            accum_out=DvRaw[:, ci:ci + 1],
                        )
                    h16s.append(h16)
                    if ci + PRE < NCHUNKS:
                        load_chunk(ci + PRE)

                rec = spool.tile([128, SC], F32, tag="rec")
                nc.vector.tensor_scalar_add(
                    rec[:], DvRaw[:, i * SC:(i + 1) * SC], EPS
                )
                nc.vector.reciprocal(rec[:], rec[:])
                nc.scalar.sqrt(Isd[:, i * SC:(i + 1) * SC], rec[:])

                xs16 = xpool.tile([128, SC, C], BF16, tag="xs")
                for t in range(SC):
                    ci = i * SC + t
                    if t % 2 == 0:
                        nc.scalar.mul(
                            xs16[:, t, :], x_t[:, t, :], Isd[:, ci:ci + 1]
                        )
                    else:
                        nc.vector.tensor_scalar_mul(
                            xs16[:, t, :], x_t[:, t, :], Isd[:, ci:ci + 1]
                        )

                # H^T built with PE transposes, staged via PSUM; the
                # staging copies carry accum_out so each copy also emits
                # the De partial for its (i, j) block. Emitted BEFORE the
                # out2T matmuls: transposes only need the casts.
                for j in range(ETILES):
                    stg = psT.tile([128, SC * 128], BF16, tag="stg")
                    for t in range(SC):
                        nc.tensor.transpose(
                            stg[:, t * 128:(t + 1) * 128],
                            h16s[t][:, j * 128:(j + 1) * 128],
                            ident16[:],
                        )
                    dest = HT3[:, j, i * SC * 128:(i + 1) * SC * 128]
                    # DVE copies are cheaper; ACT carries the 2 casts, so
                    # give DVE 9 of 16 and ACT 7.
                    if j in (0, 2, 4, 6, 8, 10, 12):
                        nc.scalar.activation(
                            dest, stg[:], COPY,
                            accum_out=DeP3[:, j, i:i + 1],
                        )
                    else:
                        nc.vector.tensor_scalar(
                            dest, stg[:], 0.0, None, ADD, ADD,
                            accum_out=DeP3[:, j, i:i + 1],
                        )

                for t in range(SC):
                    for s in range(4):
                        nc.tensor.matmul(
                            out2T_ps[:, s * 512:(s + 1) * 512],
                            xs16[:, t, :],
                            h16s[t][:, s * 512:(s + 1) * 512],
                            start=(i == 0 and t == 0),
                            stop=(i == NSUPER - 1 and t == SC - 1),
                        )

            for ci in range(PRE):
                load_chunk(ci)

            compute(0)

            # W / b prep AFTER superchunk 0 so nothing on the H critical
            # path (PE first transpose, ACT first cast) queues behind the
            # W DMA -> wt16 cast chain at startup.
            wt32 = spool.tile([128, 128], F32, tag="wt32")
            nc.gpsimd.dma_start(wt32[:], W_d.ap())
            nc.vector.tensor_copy(wt16[:], wt32[:])

            nc.gpsimd.dma_start(b_sb[:], b_d.ap())
            bb_ps = psT.tile([128, 128], F32, tag="stg", name="bb_ps")
            nc.tensor.matmul(bb_ps[:], ones1[:], b_sb[:],
                             start=True, stop=True)
            nc.scalar.copy(b_bcast[:], bb_ps[:])

            for i in range(1, NSUPER):
                compute(i)

            # --- interlude: De totals, copy out2 out of PSUM ---------------
            nc.vector.reduce_sum(RecDe[:], DeP3[:, :, :], axis=X)
            nc.vector.tensor_scalar_add(RecDe[:], RecDe[:], EPS)
            nc.vector.reciprocal(RecDe[:], RecDe[:])

            # out2T lands (bf16) in out3's buffer; each e-tile is then
            # transposed out and the scaled result overwrites it in place.
            nc.scalar.copy(out3[:, 0:1024], out2T_ps[:, 0:1024])
            nc.vector.tensor_copy(out3[:, 1024:2048], out2T_ps[:, 1024:2048])

            psA_cm.__exit__(None, None, None)

            # --- pass 2: 5 groups of column blocks (2+2+2+1+1) --------------
            # Group 0's matmul stream is interleaved with the out3 build
            # (transpose + 1/De scale per e-tile); each group's epilogue is
            # emitted after the NEXT group's stream so PE stays dense, and
            # the last group is a single block to minimize the exposed tail.
            GROUPS = [[0, 1], [2, 3], [4, 5], [6], [7]]
            psB_cm = tc.tile_pool(name="psB", bufs=4, space="PSUM")
            psB = psB_cm.__enter__()

            o4 = {}
            for grp in GROUPS:
                for blk in grp:
                    o4[blk] = psB.tile([128, 512], F32, tag="o4",
                                       name=f"o4_{blk}")

            def jstream(g):
                for j in range(ETILES):
                    for blk in GROUPS[g]:
                        nc.tensor.matmul(
                            o4[blk][:],
                            out3[:, j * 128:(j + 1) * 128],
                            HT[:, j * N + blk * 512:j * N + (blk + 1) * 512],
                            start=(j == 0), stop=(j == ETILES - 1),
                        )

            # group 0 + out3 build, interleaved per e-tile
            for j in range(ETILES):
                t2 = psT.tile([128, 128], BF16, tag="stg", name=f"t2_{j}")
                nc.tensor.transpose(
                    t2[:], out3[:, j * 128:(j + 1) * 128], ident16[:]
                )
                if j % 2 == 0:
                    nc.scalar.mul(
                        out3[:, j * 128:(j + 1) * 128], t2[:],
                        RecDe[:, j:j + 1]
                    )
                else:
                    nc.vector.tensor_scalar_mul(
                        out3[:, j * 128:(j + 1) * 128], t2[:],
                        RecDe[:, j:j + 1]
                    )
                for blk in GROUPS[0]:
                    nc.tensor.matmul(
                        o4[blk][:],
                        out3[:, j * 128:(j + 1) * 128],
                        HT[:, j * N + blk * 512:j * N + (blk + 1) * 512],
                        start=(j == 0), stop=(j == ETILES - 1),
                    )

            def epilogue(g):
                for blk in GROUPS[g]:
                    # all o4sb copies on ACT: keeps them out of the DVE
                    # queue so lp matmuls never wait behind queued stt's
                    o4sb = opool.tile([128, 512], BF16, tag="o4sb")
                    nc.scalar.copy(o4sb[:], o4[blk][:])
                    # two half-stores per block so the final DMA starts as
                    # soon as its half of the results is ready
                    for h in range(2):
                        obig = opool.tile([128, 2, C], F32, tag="obig",
                                          name=f"obig{blk}_{h}")
                        for u in range(2):
                            idx = blk * 4 + h * 2 + u
                            lp = psT.tile([128, 128], F32, tag="stg",
                                          name=f"lp_{blk}_{h}_{u}")
                            nc.tensor.matmul(
                                lp[:],
                                o4sb[:, (h * 2 + u) * 128:
                                     (h * 2 + u + 1) * 128],
                                wt16[:], start=True, stop=True,
                            )
                            nc.vector.scalar_tensor_tensor(
                                obig[:, u, :], lp[:],
                                Isd[:, idx:idx + 1], b_bcast[:],
                                mybir.AluOpType.mult, mybir.AluOpType.add,
                            )
                        r0 = blk * 512 + h * 256
                        nc.gpsimd.dma_start(
                            out_ap[r0:r0 + 256, :].rearrange(
                                "(t p) c -> p t c", p=128
                            ),
                            obig[:],
                        )

            jstream(1)
            epilogue(0)
            jstream(2)
            epilogue(1)
            jstream(3)
            epilogue(2)
            jstream(4)
            epilogue(3)
            epilogue(4)

            psB_cm.__exit__(None, None, None)
            psT_cm.__exit__(None, None, None)

    nc.compile()
    return nc


def _get_nc():
    if "nc" not in _CACHE:
        _CACHE["nc"] = _build_nc()
    return _CACHE["nc"]


def kernel(x, H, W, b):
    from concourse.bass_utils import run_bass_kernel_spmd

    nc = _get_nc()
    x = np.ascontiguousarray(x, dtype=np.float32)
    H = np.ascontiguousarray(H, dtype=np.float32)
    W = np.ascontiguousarray(W, dtype=np.float32)
    b2 = np.ascontiguousarray(b, dtype=np.float32).reshape(1, C)
    Wt = np.ascontiguousarray(W.T)
    in_maps = [
        {"x": x[c], "H": H[c], "Wt": Wt, "b": b2} for c in range(B)
    ]
    res = run_bass_kernel_spmd(nc, in_maps, core_ids=list(range(B)))
    return np.stack([res.results[c]["out"] for c in range(B)], axis=0)


# revision 10
# speedup vs baseline: 1.1087x; 1.0247x over previous
"""Trainium2 Bass kernel for a batched HGNN layer.

Per batch b (N=4096 nodes, E=2048 hyperedges, C=128 channels):
    De = sum_n H[n,e] + eps                 (hyperedge degrees)
    Dv = sum_e H[n,e] + eps                 (node degrees)
    s  = 1/sqrt(Dv)
    out = ((H @ ((H^T @ (x * s)) / De)) * s) @ W^T + b

Sharding: batch dim B=8, one batch per NeuronCore (data parallel, no
cross-core communication). Inside a core:

  pass 1 (streams H once from HBM, per-chunk 1 MiB loads on the sync
          HWDGE ring, software-pipelined):
    - fp32->bf16 casts on ACT/DVE with accum_out giving Dv row-sums
      (accum_out on the CAST is ~free; on copies it costs an extra
      ACTIVATION_READ_ACCUMULATOR (~286 ns) per op on ACT, measured)
    - H^T built with PE transposes (bf16, staged via PSUM), plain
      accum-free PSUM->SBUF staging copies split ACT/DVE
    - De partials via tiny Nf=1 ones-matmuls on PE, each paired
      directly after the transpose that shares its stationary tile
    - out2T[c,e] = (x*s)^T @ H accumulated in PSUM (PE, bf16),
      emitted AFTER the transposes so PE never waits on the
      cast->Dv->rsqrt->xs chain.
  interlude:
    - RecDe = 1/(sum De partials + eps)
    - out3[e,c] = transpose(out2T) * (1/De)
  pass 2 (H^T streamed from SBUF, no HBM traffic; 5 groups of column
          blocks so each group's epilogue overlaps the next group's
          matmul stream):
    - out4T[c,n] = out3^T @ H^T  (PE, bf16)
    - out[n,co] = (out4T_tile^T @ W^T)*s + b  (PE bf16, DVE epilogue),
      stores split in 256-row halves so the tail DMA starts early.

HBM traffic per core = 32 MB (H) + 2 MB (x) + 2 MB (out) ~= 36 MB,
i.e. the memory roofline for this problem.
"""
import os
import sys

import numpy as np

for _p in ("/opt/trn_rl_repo", "/root/.axon_site/_ro/trn_rl_repo"):
    if os.path.isdir(_p) and _p not in sys.path:
        sys.path.append(_p)

B, N, E, C = 8, 4096, 2048, 128
SC = 4                      # subchunks (128 rows) per superchunk
NSUPER = N // (128 * SC)    # 8 superchunks in pass 1
NCHUNKS = N // 128          # 32 row chunks
ETILES = E // 128           # 16 hyperedge tiles
PRE = 4                     # h32 chunk prefetch depth (1 MiB each)
EPS = 1e-6

_CACHE = {}


def _build_nc():
    from contextlib import ExitStack

    import concourse.tile as tile
    from concourse import bacc, mybir

    F32 = mybir.dt.float32
    BF16 = mybir.dt.bfloat16
    X = mybir.AxisListType.X
    COPY = mybir.ActivationFunctionType.Copy
    ADD = mybir.AluOpType.add

    nc = bacc.Bacc("TRN2", target_bir_lowering=False, debug=False)

    H_d = nc.dram_tensor("H", [N, E], F32, kind="ExternalInput")
    x_d = nc.dram_tensor("x", [N, C], F32, kind="ExternalInput")
    W_d = nc.dram_tensor("Wt", [C, C], F32, kind="ExternalInput")
    b_d = nc.dram_tensor("b", [1, C], F32, kind="ExternalInput")
    out_d = nc.dram_tensor("out", [N, C], F32, kind="ExternalOutput")

    H_ap, x_ap, out_ap = H_d.ap(), x_d.ap(), out_d.ap()

    with tile.TileContext(nc) as tc:
        with ExitStack() as ctx:
            const = ctx.enter_context(tc.tile_pool(name="const", bufs=1))
            h32p = ctx.enter_context(tc.tile_pool(name="h32", bufs=PRE))
            h16p = ctx.enter_context(tc.tile_pool(name="h16", bufs=6))
            xpool = ctx.enter_context(tc.tile_pool(name="xp", bufs=2))
            spool = ctx.enter_context(tc.tile_pool(name="sp", bufs=2))
            opool = ctx.enter_context(tc.tile_pool(name="op", bufs=2))
            psT_cm = tc.tile_pool(name="psT", bufs=4, space="PSUM")
            psT = psT_cm.__enter__()
            psA_cm = tc.tile_pool(name="psA", bufs=1, space="PSUM")
            psA = psA_cm.__enter__()

            # --- constants -------------------------------------------------
            ident16 = const.tile([128, 128], BF16)
            nc.vector.memset(ident16[:], 1.0)
            nc.gpsimd.affine_select(
                ident16[:], ident16[:], pattern=[[-1, 128]], base=0,
                channel_multiplier=1, compare_op=mybir.AluOpType.is_equal,
                fill=0.0,
            )
            ones1 = const.tile([1, 128], F32)
            nc.vector.memset(ones1[:], 1.0)
            ones_n = const.tile([128, 1], BF16)
            nc.vector.memset(ones_n[:], 1.0)

            # --- persistent state ------------------------------------------
            HT = const.tile([128, ETILES * N], BF16)     # H^T cache, 128 KB/part
            out3 = const.tile([128, ETILES * 128], BF16)  # (H^T xs)/De, [e, c]
            Isd = const.tile([128, NCHUNKS], F32)        # 1/sqrt(Dv)
            DvRaw = const.tile([128, NCHUNKS], F32)
            DeP2 = const.tile([128, ETILES * NSUPER], F32)  # De partials
            RecDe = const.tile([128, ETILES], F32)
            wt16 = const.tile([128, 128], BF16)          # W^T: [c_in, c_out]
            b_bcast = const.tile([128, 128], F32)        # b replicated per row
            b_sb = const.tile([1, 128], F32)

            out2T_ps = psA.tile([128, E], F32)           # 4 PSUM banks

            HT3 = HT[:].rearrange("p (j n) -> p j n", j=ETILES)
            DeP3 = DeP2[:].rearrange("p (j i) -> p j i", j=ETILES)

            h32_tiles = {}

            def load_chunk(ci):
                """DMA one 128-row chunk of H (1 MiB fp32) on the sync ring.

                The sync engine is otherwise idle so triggers issue
                immediately; per-chunk granularity lets the first cast
                start ~3 us after kernel start and keeps buffer recycling
                fine-grained so the 16 HW DMA engines stay dense.
                """
                h32 = h32p.tile([128, E], F32, tag="h32", name=f"h32_{ci}")
                nc.sync.dma_start(h32[:], H_ap[ci * 128:(ci + 1) * 128, :])
                h32_tiles[ci] = h32

            def compute(i):
                x_t = xpool.tile([128, SC, C], F32, tag="x")
                nc.gpsimd.dma_start(
                    x_t[:],
                    x_ap[i * SC * 128:(i + 1) * SC * 128, :].rearrange(
                        "(t p) c -> p t c", p=128
                    ),
                )
                # fp32->bf16 casts with Dv row-sums fused via accum_out,
                # alternating ACT/DVE. Each cast frees an h32 buffer, so
                # the prefetch of chunk ci+PRE is emitted right after it.
                h16s = []
                for t in range(SC):
                    ci = i * SC + t
                    h16 = h16p.tile([128, E], BF16, tag="h16",
                                    name=f"h16_{i}_{t}")
                    h32 = h32_tiles.pop(ci)
                    if t % 2 == 0:
                        nc.scalar.activation(
                            h16[:], h32[:], COPY,
                            accum_out=DvRaw[:, ci:ci + 1],
                        )
                    else:
                        nc.vector.tensor_scalar(
                            h16[:], h32[:], 0.0, None, ADD, ADD,
                            accum_out=DvRaw[:, ci:ci + 1],
                        )
                    h16s.append(h16)
                    if ci + PRE < NCHUNKS:
                        load_chunk(ci + PRE)

                rec = spool.tile([128, SC], F32, tag="rec")
                nc.vector.tensor_scalar_add(
                    rec[:], DvRaw[:, i * SC:(i + 1) * SC], EPS
                )
                nc.vector.reciprocal(rec[:], rec[:])
                nc.scalar.sqrt(Isd[:, i * SC:(i + 1) * SC], rec[:])

                xs16 = xpool.tile([128, SC, C], BF16, tag="xs")
                for t in range(SC):
                    ci = i * SC + t
                    if t % 2 == 0:
                        nc.scalar.mul(
                            xs16[:, t, :], x_t[:, t, :], Isd[:, ci:ci + 1]
                        )
                    else:
                        nc.vector.tensor_scalar_mul(
                            xs16[:, t, :], x_t[:, t, :], Isd[:, ci:ci + 1]
                        )

                # H^T built with PE transposes, staged via PSUM, with
                # plain accum-free copies to the SBUF cache. De partials
                # via tiny Nf=1 ones-matmuls, each paired right after the
                # transpose sharing its stationary tile. Emitted BEFORE
                # the out2T matmuls: transposes only need the casts.
                deps = psT.tile([128, ETILES], F32, tag="stg",
                                name=f"deps_{i}")
                for j in range(ETILES):
                    stg = psT.tile([128, SC * 128], BF16, tag="stg")
                    for t in range(SC):
                        nc.tensor.transpose(
                            stg[:, t * 128:(t + 1) * 128],
                            h16s[t][:, j * 128:(j + 1) * 128],
                            ident16[:],
                        )
                        nc.tensor.matmul(
                            deps[:, j:j + 1],
                            h16s[t][:, j * 128:(j + 1) * 128],
                            ones_n[:], start=(t == 0), stop=(t == SC - 1),
                        )
                    dest = HT3[:, j, i * SC * 128:(i + 1) * SC * 128]
                    # DVE copies are cheaper; ACT carries the 2 casts, so
                    # give DVE 9 of 16 and ACT 7.
                    if j in (0, 2, 4, 6, 8, 10, 12):
                        nc.scalar.copy(dest, stg[:])
                    else:
                        nc.vector.tensor_copy(dest, stg[:])
                nc.scalar.copy(DeP3[:, :, i:i + 1], deps[:].rearrange(
                    "p (j o) -> p j o", o=1))

                for t in range(SC):
                    for s in range(4):
                        nc.tensor.matmul(
                            out2T_ps[:, s * 512:(s + 1) * 512],
                            xs16[:, t, :],
                            h16s[t][:, s * 512:(s + 1) * 512],
                            start=(i == 0 and t == 0),
                            stop=(i == NSUPER - 1 and t == SC - 1),
                        )

            for ci in range(PRE):
                load_chunk(ci)

            compute(0)

            # W / b prep AFTER superchunk 0 so nothing on the H critical
            # path (PE first transpose, ACT first cast) queues behind the
            # W DMA -> wt16 cast chain at startup.
            wt32 = spool.tile([128, 128], F32, tag="wt32")
            nc.gpsimd.dma_start(wt32[:], W_d.ap())
            nc.vector.tensor_copy(wt16[:], wt32[:])

            nc.gpsimd.dma_start(b_sb[:], b_d.ap())
            bb_ps = psT.tile([128, 128], F32, tag="stg", name="bb_ps")
            nc.tensor.matmul(bb_ps[:], ones1[:], b_sb[:],
                             start=True, stop=True)
            nc.scalar.copy(b_bcast[:], bb_ps[:])

            for i in range(1, NSUPER):
                compute(i)

            # --- interlude: De totals, copy out2 out of PSUM ---------------
            nc.vector.reduce_sum(RecDe[:], DeP3[:, :, :], axis=X)
            nc.vector.tensor_scalar_add(RecDe[:], RecDe[:], EPS)
            nc.vector.reciprocal(RecDe[:], RecDe[:])

            # out2T lands (bf16) in out3's buffer; each e-tile is then
            # transposed out and the scaled result overwrites it in place.
            nc.scalar.copy(out3[:, 0:1024], out2T_ps[:, 0:1024])
            nc.vector.tensor_copy(out3[:, 1024:2048], out2T_ps[:, 1024:2048])

            psA_cm.__exit__(None, None, None)

            # --- pass 2: 5 groups of column blocks (2+2+2+1+1) --------------
            # Group 0's matmul stream is interleaved with the out3 build
            # (transpose + 1/De scale per e-tile); each group's epilogue is
            # emitted after the NEXT group's stream so PE stays dense, and
            # the last group is a single block to minimize the exposed tail.
            GROUPS = [[0, 1], [2, 3], [4, 5], [6], [7]]
            psB_cm = tc.tile_pool(name="psB", bufs=4, space="PSUM")
            psB = psB_cm.__enter__()

            o4 = {}
            for grp in GROUPS:
                for blk in grp:
                    o4[blk] = psB.tile([128, 512], F32, tag="o4",
                                       name=f"o4_{blk}")

            def jstream(g):
                for j in range(ETILES):
                    for blk in GROUPS[g]:
                        nc.tensor.matmul(
                            o4[blk][:],
                            out3[:, j * 128:(j + 1) * 128],
                            HT[:, j * N + blk * 512:j * N + (blk + 1) * 512],
                            start=(j == 0), stop=(j == ETILES - 1),
                        )

            # group 0 + out3 build, interleaved per e-tile
            for j in range(ETILES):
                t2 = psT.tile([128, 128], BF16, tag="stg", name=f"t2_{j}")
                nc.tensor.transpose(
                    t2[:], out3[:, j * 128:(j + 1) * 128], ident16[:]
                )
                if j % 2 == 0:
                    nc.scalar.mul(
                        out3[:, j * 128:(j + 1) * 128], t2[:],
                        RecDe[:, j:j + 1]
                    )
                else:
                    nc.vector.tensor_scalar_mul(
                        out3[:, j * 128:(j + 1) * 128], t2[:],
                        RecDe[:, j:j + 1]
                    )
                for blk in GROUPS[0]:
                    nc.tensor.matmul(
                        o4[blk][:],
                        out3[:, j * 128:(j + 1) * 128],
                        HT[:, j * N + blk * 512:j * N + (blk + 1) * 512],
                        start=(j == 0), stop=(j == ETILES - 1),
                    )

            def epilogue(g):
                for blk in GROUPS[g]:
                    # all o4sb copies on ACT: keeps them out of the DVE
                    # queue so lp matmuls never wait behind queued stt's
                    o4sb = opool.tile([128, 512], BF16, tag="o4sb")
                    nc.scalar.copy(o4sb[:], o4[blk][:])
                    # two half-stores per block so the final DMA starts as
                    # soon as its half of the results is ready
                    for h in range(2):
                        obig = opool.tile([128, 2, C], F32, tag="obig",
                                          name=f"obig{blk}_{h}")
                        for u in range(2):
                            idx = blk * 4 + h * 2 + u
                            lp = psT.tile([128, 128], F32, tag="stg",
                                          name=f"lp_{blk}_{h}_{u}")
                            nc.tensor.matmul(
                                lp[:],
                                o4sb[:, (h * 2 + u) * 128:
                                     (h * 2 + u + 1) * 128],
                                wt16[:], start=True, stop=True,
                            )
                            nc.vector.scalar_tensor_tensor(
                                obig[:, u, :], lp[:],
                                Isd[:, idx:idx + 1], b_bcast[:],
                                mybir.AluOpType.mult, mybir.AluOpType.add,
                            )
                        r0 = blk * 512 + h * 256
                        nc.gpsimd.dma_start(
                            out_ap[r0:r0 + 256, :].rearrange(
                                "(t p) c -> p t c", p=128
                            ),
                            obig[:],
                        )

            jstream(1)
            epilogue(0)
            jstream(2)
            epilogue(1)
            jstream(3)
            epilogue(2)
            jstream(4)
            epilogue(3)
            epilogue(4)

            psB_cm.__exit__(None, None, None)
            psT_cm.__exit__(None, None, None)

    nc.compile()
    return nc


def _get_nc():
    if "nc" not in _CACHE:
        _CACHE["nc"] = _build_nc()
    return _CACHE["nc"]


def kernel(x, H, W, b):
    from concourse.bass_utils import run_bass_kernel_spmd

    nc = _get_nc()
    x = np.ascontiguousarray(x, dtype=np.float32)
    H = np.ascontiguousarray(H, dtype=np.float32)
    W = np.ascontiguousarray(W, dtype=np.float32)
    b2 = np.ascontiguousarray(b, dtype=np.float32).reshape(1, C)
    Wt = np.ascontiguousarray(W.T)
    in_maps = [
        {"x": x[c], "H": H[c], "Wt": Wt, "b": b2} for c in range(B)
    ]
    res = run_bass_kernel_spmd(nc, in_maps, core_ids=list(range(B)))
    return np.stack([res.results[c]["out"] for c in range(B)], axis=0)


# revision 15
# speedup vs baseline: 1.1607x; 1.0470x over previous
"""Trainium2 Bass kernel for a batched HGNN layer.

Per batch b (N=4096 nodes, E=2048 hyperedges, C=128 channels):
    De = sum_n H[n,e] + eps                 (hyperedge degrees)
    Dv = sum_e H[n,e] + eps                 (node degrees)
    s  = 1/sqrt(Dv)
    out = ((H @ ((H^T @ (x * s)) / De)) * s) @ W^T + b

Sharding: batch dim B=8, one batch per NeuronCore (data parallel, no
cross-core communication). Inside a core:

  pass 1 (streams H once from HBM in 8 superchunks of 512 rows,
          software-pipelined, fp32->bf16 cast done by the SWDGE DMA
          itself so ACT/DVE only do copies/reductions):
    - Dv row-sums via DVE reduce on the bf16 tile
    - out2T[c,e] = (x*s)^T @ H accumulated in PSUM (PE, bf16)
    - H^T built with PE transposes, staged via PSUM, copied to a
      16 MB bf16 SBUF cache with plain ACT/DVE copies (no accum_out)
    - De col-sums via grouped DVE reduces over the fresh H^T slices
  interlude (fused into pass 2's first block-pair stream):
    - out3[e,c] = transpose(out2T) * (1/De)
  pass 2 (H^T streamed from SBUF, no HBM traffic; 4 groups of 2
          column blocks so each group's epilogue overlaps the next
          group's matmul stream):
    - out4T[c,n] = out3^T @ H^T  (PE, bf16)
    - out[n,co] = (out4T_tile^T @ W^T)*s + b  (PE bf16, DVE epilogue)

HBM traffic per core = 32 MB (H) + 2 MB (x) + 2 MB (out) ~= 36 MB,
i.e. the memory roofline for this problem.
"""
import os
import sys

import numpy as np

for _p in ("/opt/trn_rl_repo", "/root/.axon_site/_ro/trn_rl_repo"):
    if os.path.isdir(_p) and _p not in sys.path:
        sys.path.append(_p)

B, N, E, C = 8, 4096, 2048, 128
SC = 4                      # subchunks (128 rows) per superchunk
KHT = 130                   # H^T cache block: 128 cols + De partial + pad
                            # (130 keeps each PSUM transpose dest 4B-aligned)
NSUPER = N // (128 * SC)    # 8 superchunks in pass 1
NCHUNKS = N // 128          # 32 row chunks
ETILES = E // 128           # 16 hyperedge tiles
EPS = 1e-6

_CACHE = {}


def _build_nc():
    from contextlib import ExitStack

    import concourse.tile as tile
    from concourse import bacc, mybir

    F32 = mybir.dt.float32
    BF16 = mybir.dt.bfloat16
    X = mybir.AxisListType.X
    XY = mybir.AxisListType.XY
    COPY = mybir.ActivationFunctionType.Copy
    ADD = mybir.AluOpType.add

    nc = bacc.Bacc("TRN2", target_bir_lowering=False, debug=False)

    H_d = nc.dram_tensor("H", [N, E], F32, kind="ExternalInput")
    x_d = nc.dram_tensor("x", [N, C], F32, kind="ExternalInput")
    W_d = nc.dram_tensor("Wt", [C, C], F32, kind="ExternalInput")
    b_d = nc.dram_tensor("b", [1, C], F32, kind="ExternalInput")
    out_d = nc.dram_tensor("out", [N, C], F32, kind="ExternalOutput")

    H_ap, x_ap, out_ap = H_d.ap(), x_d.ap(), out_d.ap()

    with tile.TileContext(nc) as tc:
        with ExitStack() as ctx:
            const = ctx.enter_context(tc.tile_pool(name="const", bufs=1))
            h32p = ctx.enter_context(tc.tile_pool(name="h32", bufs=2))
            h16p = ctx.enter_context(tc.tile_pool(name="h16", bufs=6))
            xpool = ctx.enter_context(tc.tile_pool(name="xp", bufs=2))
            spool = ctx.enter_context(tc.tile_pool(name="sp", bufs=2))
            opool = ctx.enter_context(tc.tile_pool(name="op", bufs=2))
            psT_cm = tc.tile_pool(name="psT", bufs=4, space="PSUM")
            psT = psT_cm.__enter__()
            psA_cm = tc.tile_pool(name="psA", bufs=1, space="PSUM")
            psA = psA_cm.__enter__()

            # --- constants -------------------------------------------------
            # Extended identity [I | 1 1]: transposing with it makes columns
            # 128/129 of each PE transpose the partition-sum of the input
            # tile, i.e. a free De (hyperedge-degree) partial -- no
            # accum_out needed on the staging copies.
            ident16 = const.tile([128, 128], BF16)
            nc.vector.memset(ident16[:], 1.0)
            nc.gpsimd.affine_select(
                ident16[:], ident16[:], pattern=[[-1, 128]], base=0,
                channel_multiplier=1, compare_op=mybir.AluOpType.is_equal,
                fill=0.0,
            )
            ones_n = const.tile([128, 1], BF16)
            nc.vector.memset(ones_n[:], 1.0)

            # --- persistent state ------------------------------------------
            HT = const.tile([128, ETILES * N], BF16)     # H^T cache, 128 KB/part
            out3 = const.tile([128, ETILES * 128], BF16)  # (H^T xs)/De, [e, c]
            Isd = const.tile([128, NCHUNKS], F32)        # 1/sqrt(Dv)
            DvRaw = const.tile([128, NCHUNKS], F32)
            DeP2 = const.tile([128, ETILES * NSUPER], F32)  # De partials
            RecDe = const.tile([128, ETILES], F32)

            out2T_ps = psA.tile([128, E], F32)           # 4 PSUM banks

            HT3 = HT[:].rearrange("p (j n) -> p j n", j=ETILES)
            DeP3 = DeP2[:].rearrange("p (j i) -> p j i", j=ETILES)

            # --- pass 1 (software pipelined) -------------------------------
            def load(i):
                """DMA superchunk i (fp32) on the sync HWDGE ring.

                All H chunks go on nc.sync: the sync engine is otherwise
                idle, so triggers issue immediately.  (nc.scalar triggers
                sit in the busy ACT queue and stall the stream.)  2 MiB per
                call (2 row-chunks) for better DMA efficiency and fewer
                completion gaps.
                """
                h32s = []
                for h in range(SC // 2):
                    h32 = h32p.tile([128, 2, E], F32, tag="h32")
                    r0 = (i * SC + 2 * h) * 128
                    if i == 0:
                        # superchunk 0 in per-chunk 1 MiB calls: separate
                        # completion semaphores let the first cast (and so
                        # the first PE transpose) start ~3 us earlier.
                        for u in range(2):
                            nc.sync.dma_start(
                                h32[:, u, :],
                                H_ap[r0 + u * 128:r0 + (u + 1) * 128, :],
                            )
                    else:
                        nc.sync.dma_start(
                            h32[:],
                            H_ap[r0:r0 + 256, :].rearrange("(t p) e -> p t e",
                                                           p=128),
                        )
                    h32s.append(h32[:, 0, :])
                    h32s.append(h32[:, 1, :])
                return h32s

            def compute(i, h32s):
                x_t = xpool.tile([128, SC, C], F32, tag="x")
                nc.gpsimd.dma_start(
                    x_t[:],
                    x_ap[i * SC * 128:(i + 1) * SC * 128, :].rearrange(
                        "(t p) c -> p t c", p=128
                    ),
                )
                # fp32->bf16 casts with Dv row-sums fused via accum_out,
                # alternating ACT/DVE (explicit TensorReduce has no fast
                # DVE mode, so fusion is the only affordable reduction).
                h16s = []
                for t in range(SC):
                    ci = i * SC + t
                    h16 = h16p.tile([128, E], BF16, tag="h16",
                                    name=f"h16_{i}_{t}")
                    if t % 2 == 0:
                        nc.scalar.activation(
                            h16[:], h32s[t], COPY,
                            accum_out=DvRaw[:, ci:ci + 1],
                        )
                    else:
                        nc.vector.tensor_scalar(
                            h16[:], h32s[t], 0.0, None, ADD, ADD,
                            accum_out=DvRaw[:, ci:ci + 1],
                        )
                    h16s.append(h16)
                rec = spool.tile([128, SC], F32, tag="rec")
                nc.vector.tensor_scalar_add(
                    rec[:], DvRaw[:, i * SC:(i + 1) * SC], EPS
                )
                nc.vector.reciprocal(rec[:], rec[:])
                nc.scalar.sqrt(Isd[:, i * SC:(i + 1) * SC], rec[:])

                xs16 = xpool.tile([128, SC, C], BF16, tag="xs")
                for t in range(SC):
                    ci = i * SC + t
                    if t % 2 == 0:
                        nc.scalar.mul(
                            xs16[:, t, :], x_t[:, t, :], Isd[:, ci:ci + 1]
                        )
                    else:
                        nc.vector.tensor_scalar_mul(
                            xs16[:, t, :], x_t[:, t, :], Isd[:, ci:ci + 1]
                        )

                # De partials for this superchunk on the PE: tiny Nf=1
                # ones-matmuls accumulated over the 4 chunks per e-tile.
                # Keeps the staging copies accum_out-free (the DVE/ACT
                # fused-reduce tax was the pass-1 bottleneck). The whole
                # j-loop is emitted BEFORE the out2T matmuls: transposes
                # need only the casts, not the Dv->rsqrt->xs chain, so PE
                # starts earlier at each superchunk boundary.
                deps = psT.tile([128, ETILES], F32, tag="stg",
                                name=f"deps_{i}")
                for j in range(ETILES):
                    stg = psT.tile([128, SC * 128], BF16, tag="stg")
                    for t in range(SC):
                        nc.tensor.transpose(
                            stg[:, t * 128:(t + 1) * 128],
                            h16s[t][:, j * 128:(j + 1) * 128],
                            ident16[:],
                        )
                    for t in range(SC):
                        nc.tensor.matmul(
                            deps[:, j:j + 1],
                            h16s[t][:, j * 128:(j + 1) * 128],
                            ones_n[:], start=(t == 0), stop=(t == SC - 1),
                        )
                    dest = HT3[:, j, i * SC * 128:(i + 1) * SC * 128]
                    # 7 copies on ACT, 9 on DVE (ACT also carries 2 casts
                    # and is the tighter engine)
                    if j % 2 == 0 and j != 14:
                        nc.scalar.copy(dest, stg[:])
                    else:
                        nc.vector.tensor_copy(dest, stg[:])
                nc.scalar.copy(DeP3[:, :, i:i + 1], deps[:].rearrange(
                    "p (j o) -> p j o", o=1))

                for t in range(SC):
                    for s in range(4):
                        nc.tensor.matmul(
                            out2T_ps[:, s * 512:(s + 1) * 512],
                            xs16[:, t, :],
                            h16s[t][:, s * 512:(s + 1) * 512],
                            start=(i == 0 and t == 0),
                            stop=(i == NSUPER - 1 and t == SC - 1),
                        )

            h32s_cur = load(0)
            h32s_next = load(1)
            compute(0, h32s_cur)
            h32s_cur = h32s_next

            # W / b prep AFTER superchunk 0's compute: nothing on the H
            # critical path (first casts, first transposes) then queues
            # behind the W DMA -> wt16 cast chain at startup.
            wt32 = spool.tile([128, 128], F32, tag="wt32")
            nc.gpsimd.dma_start(wt32[:], W_d.ap())
            wt16 = const.tile([128, 128], BF16)          # W^T: [c_in, c_out]
            nc.vector.tensor_copy(wt16[:], wt32[:])

            b_sb = const.tile([1, 128], F32)
            nc.gpsimd.dma_start(b_sb[:], b_d.ap())
            ones1 = const.tile([1, 128], F32)
            nc.vector.memset(ones1[:], 1.0)
            bb_ps = psT.tile([128, 128], F32, tag="stg")
            nc.tensor.matmul(bb_ps[:], ones1[:], b_sb[:], start=True, stop=True)
            b_bcast = const.tile([128, 128], F32)        # b replicated per row
            nc.scalar.copy(b_bcast[:], bb_ps[:])

            for i in range(1, NSUPER):
                h32s_next = load(i + 1) if i + 1 < NSUPER else None
                compute(i, h32s_cur)
                h32s_cur = h32s_next

            # --- interlude: De totals, copy out2 out of PSUM ---------------
            nc.vector.reduce_sum(RecDe[:], DeP3[:, :, :], axis=X)
            nc.vector.tensor_scalar_add(RecDe[:], RecDe[:], EPS)
            nc.vector.reciprocal(RecDe[:], RecDe[:])

            # out2T lands (bf16) in out3's buffer; each e-tile is then
            # transposed out and the scaled result overwrites it in place.
            nc.scalar.copy(out3[:, 0:1024], out2T_ps[:, 0:1024])
            nc.vector.tensor_copy(out3[:, 1024:2048], out2T_ps[:, 1024:2048])

            psA_cm.__exit__(None, None, None)

            # --- pass 2: 4 groups of column blocks (3+2+2+1) ---------------
            # Group 0's matmul stream is interleaved with the out3 build
            # (transpose + 1/De scale per e-tile); each group's epilogue is
            # emitted after the NEXT group's stream so PE stays dense, and
            # the last group is a single block to minimize the exposed tail.
            GROUPS = [[0, 1], [2, 3], [4, 5], [6], [7]]
            psB_cm = tc.tile_pool(name="psB", bufs=4, space="PSUM")
            psB = psB_cm.__enter__()

            o4 = {}
            for grp in GROUPS:
                for blk in grp:
                    o4[blk] = psB.tile([128, 512], F32, tag="o4",
                                       name=f"o4_{blk}")

            def jstream(g):
                for j in range(ETILES):
                    for blk in GROUPS[g]:
                        nc.tensor.matmul(
                            o4[blk][:],
                            out3[:, j * 128:(j + 1) * 128],
                            HT[:, j * N + blk * 512:j * N + (blk + 1) * 512],
                            start=(j == 0), stop=(j == ETILES - 1),
                        )

            # group 0 + out3 build, interleaved per e-tile
            for j in range(ETILES):
                t2 = psT.tile([128, 128], BF16, tag="stg")
                nc.tensor.transpose(
                    t2[:], out3[:, j * 128:(j + 1) * 128], ident16[:]
                )
                if j % 2 == 0:
                    nc.scalar.mul(
                        out3[:, j * 128:(j + 1) * 128], t2[:],
                        RecDe[:, j:j + 1]
                    )
                else:
                    nc.vector.tensor_scalar_mul(
                        out3[:, j * 128:(j + 1) * 128], t2[:],
                        RecDe[:, j:j + 1]
                    )
                for blk in GROUPS[0]:
                    nc.tensor.matmul(
                        o4[blk][:],
                        out3[:, j * 128:(j + 1) * 128],
                        HT[:, j * N + blk * 512:j * N + (blk + 1) * 512],
                        start=(j == 0), stop=(j == ETILES - 1),
                    )

            def epilogue(g, halves=False):
                for blk in GROUPS[g]:
                    # all o4sb copies on ACT: keeps them out of the DVE
                    # queue so lp matmuls never wait behind queued stt's
                    o4sb = opool.tile([128, 512], BF16, tag="o4sb")
                    nc.scalar.copy(o4sb[:], o4[blk][:])
                    # tail blocks store in 256-row halves so the last DMA
                    # starts as soon as its half of the results is ready
                    nh = 2 if halves else 1
                    for h in range(nh):
                        obig = opool.tile([128, 4 // nh, C], F32,
                                          tag="obig", name=f"obig{blk}_{h}")
                        for u in range(4 // nh):
                            t = h * (4 // nh) + u
                            idx = blk * 4 + t
                            lp = psT.tile([128, 128], F32, tag="stg",
                                          name=f"lp_{blk}_{t}")
                            nc.tensor.matmul(
                                lp[:], o4sb[:, t * 128:(t + 1) * 128],
                                wt16[:], start=True, stop=True,
                            )
                            nc.vector.scalar_tensor_tensor(
                                obig[:, u, :], lp[:], Isd[:, idx:idx + 1],
                                b_bcast[:],
                                mybir.AluOpType.mult, mybir.AluOpType.add,
                            )
                        r0 = blk * 512 + h * (512 // nh)
                        nc.gpsimd.dma_start(
                            out_ap[r0:r0 + 512 // nh, :].rearrange(
                                "(t p) c -> p t c", p=128
                            ),
                            obig[:],
                        )

            jstream(1)
            epilogue(0)
            jstream(2)
            epilogue(1)
            jstream(3)
            epilogue(2)
            jstream(4)
            epilogue(3, halves=True)
            epilogue(4, halves=True)

            psB_cm.__exit__(None, None, None)
            psT_cm.__exit__(None, None, None)

    nc.compile()
    return nc


def _get_nc():
    if "nc" not in _CACHE:
        _CACHE["nc"] = _build_nc()
    return _CACHE["nc"]


def kernel(x, H, W, b):
    from concourse.bass_utils import run_bass_kernel_spmd

    nc = _get_nc()
    x = np.ascontiguousarray(x, dtype=np.float32)
    H = np.ascontiguousarray(H, dtype=np.float32)
    W = np.ascontiguousarray(W, dtype=np.float32)
    b2 = np.ascontiguousarray(b, dtype=np.float32).reshape(1, C)
    Wt = np.ascontiguousarray(W.T)
    in_maps = [
        {"x": x[c], "H": H[c], "Wt": Wt, "b": b2} for c in range(B)
    ]
    res = run_bass_kernel_spmd(nc, in_maps, core_ids=list(range(B)))
    return np.stack([res.results[c]["out"] for c in range(B)], axis=0)



# revision 16
# speedup vs baseline: 1.1786x; 1.0154x over previous
"""Trainium2 Bass kernel for a batched HGNN layer.

Per batch b (N=4096 nodes, E=2048 hyperedges, C=128 channels):
    De = sum_n H[n,e] + eps                 (hyperedge degrees)
    Dv = sum_e H[n,e] + eps                 (node degrees)
    s  = 1/sqrt(Dv)
    out = ((H @ ((H^T @ (x * s)) / De)) * s) @ W^T + b

Sharding: batch dim B=8, one batch per NeuronCore (data parallel, no
cross-core communication). Inside a core:

  pass 1 (streams H once from HBM in 8 superchunks of 512 rows,
          software-pipelined, fp32->bf16 cast done by the SWDGE DMA
          itself so ACT/DVE only do copies/reductions):
    - Dv row-sums via DVE reduce on the bf16 tile
    - out2T[c,e] = (x*s)^T @ H accumulated in PSUM (PE, bf16)
    - H^T built with PE transposes, staged via PSUM, copied to a
      16 MB bf16 SBUF cache with plain ACT/DVE copies (no accum_out)
    - De col-sums via grouped DVE reduces over the fresh H^T slices
  interlude (fused into pass 2's first block-pair stream):
    - out3[e,c] = transpose(out2T) * (1/De)
  pass 2 (H^T streamed from SBUF, no HBM traffic; 4 groups of 2
          column blocks so each group's epilogue overlaps the next
          group's matmul stream):
    - out4T[c,n] = out3^T @ H^T  (PE, bf16)
    - out[n,co] = (out4T_tile^T @ W^T)*s + b  (PE bf16, DVE epilogue)

HBM traffic per core = 32 MB (H) + 2 MB (x) + 2 MB (out) ~= 36 MB,
i.e. the memory roofline for this problem.
"""
import os
import sys

import numpy as np

for _p in ("/opt/trn_rl_repo", "/root/.axon_site/_ro/trn_rl_repo"):
    if os.path.isdir(_p) and _p not in sys.path:
        sys.path.append(_p)

B, N, E, C = 8, 4096, 2048, 128
SC = 4                      # subchunks (128 rows) per superchunk
KHT = 130                   # H^T cache block: 128 cols + De partial + pad
                            # (130 keeps each PSUM transpose dest 4B-aligned)
NSUPER = N // (128 * SC)    # 8 superchunks in pass 1
NCHUNKS = N // 128          # 32 row chunks
ETILES = E // 128           # 16 hyperedge tiles
EPS = 1e-6

_CACHE = {}


def _build_nc():
    from contextlib import ExitStack

    import concourse.tile as tile
    from concourse import bacc, mybir

    F32 = mybir.dt.float32
    BF16 = mybir.dt.bfloat16
    X = mybir.AxisListType.X
    XY = mybir.AxisListType.XY
    COPY = mybir.ActivationFunctionType.Copy
    ADD = mybir.AluOpType.add

    nc = bacc.Bacc("TRN2", target_bir_lowering=False, debug=False)

    H_d = nc.dram_tensor("H", [N, E], F32, kind="ExternalInput")
    x_d = nc.dram_tensor("x", [N, C], F32, kind="ExternalInput")
    W_d = nc.dram_tensor("Wt", [C, C], F32, kind="ExternalInput")
    b_d = nc.dram_tensor("b", [1, C], F32, kind="ExternalInput")
    out_d = nc.dram_tensor("out", [N, C], F32, kind="ExternalOutput")

    H_ap, x_ap, out_ap = H_d.ap(), x_d.ap(), out_d.ap()

    with tile.TileContext(nc) as tc:
        with ExitStack() as ctx:
            const = ctx.enter_context(tc.tile_pool(name="const", bufs=1))
            h32p = ctx.enter_context(tc.tile_pool(name="h32", bufs=2))
            h16p = ctx.enter_context(tc.tile_pool(name="h16", bufs=6))
            xpool = ctx.enter_context(tc.tile_pool(name="xp", bufs=2))
            spool = ctx.enter_context(tc.tile_pool(name="sp", bufs=2))
            opool = ctx.enter_context(tc.tile_pool(name="op", bufs=2))
            psT_cm = tc.tile_pool(name="psT", bufs=4, space="PSUM")
            psT = psT_cm.__enter__()
            psA_cm = tc.tile_pool(name="psA", bufs=1, space="PSUM")
            psA = psA_cm.__enter__()

            # --- constants -------------------------------------------------
            # Extended identity [I | 1 1]: transposing with it makes columns
            # 128/129 of each PE transpose the partition-sum of the input
            # tile, i.e. a free De (hyperedge-degree) partial -- no
            # accum_out needed on the staging copies.
            ident16 = const.tile([128, 128], BF16)
            nc.vector.memset(ident16[:], 1.0)
            nc.gpsimd.affine_select(
                ident16[:], ident16[:], pattern=[[-1, 128]], base=0,
                channel_multiplier=1, compare_op=mybir.AluOpType.is_equal,
                fill=0.0,
            )
            ones_n = const.tile([128, 1], BF16)
            nc.vector.memset(ones_n[:], 1.0)

            # --- persistent state ------------------------------------------
            HT = const.tile([128, ETILES * N], BF16)     # H^T cache, 128 KB/part
            out3 = const.tile([128, ETILES * 128], BF16)  # (H^T xs)/De, [e, c]
            Isd = const.tile([128, NCHUNKS], F32)        # 1/sqrt(Dv)
            DvRaw = const.tile([128, NCHUNKS], F32)
            DeP2 = const.tile([128, ETILES * NSUPER], F32)  # De partials
            RecDe = const.tile([128, ETILES], F32)

            out2T_ps = psA.tile([128, E], F32)           # 4 PSUM banks

            HT3 = HT[:].rearrange("p (j n) -> p j n", j=ETILES)
            DeP3 = DeP2[:].rearrange("p (j i) -> p j i", j=ETILES)

            # --- pass 1 (software pipelined) -------------------------------
            def load(i):
                """DMA superchunk i (fp32) on the sync HWDGE ring.

                All H chunks go on nc.sync: the sync engine is otherwise
                idle, so triggers issue immediately.  (nc.scalar triggers
                sit in the busy ACT queue and stall the stream.)  2 MiB per
                call (2 row-chunks) for better DMA efficiency and fewer
                completion gaps.
                """
                h32s = []
                for h in range(SC // 2):
                    h32 = h32p.tile([128, 2, E], F32, tag="h32")
                    r0 = (i * SC + 2 * h) * 128
                    if i == 0:
                        # superchunk 0 in per-chunk 1 MiB calls: separate
                        # completion semaphores let the first cast (and so
                        # the first PE transpose) start ~3 us earlier.
                        for u in range(2):
                            nc.sync.dma_start(
                                h32[:, u, :],
                                H_ap[r0 + u * 128:r0 + (u + 1) * 128, :],
                            )
                    else:
                        nc.sync.dma_start(
                            h32[:],
                            H_ap[r0:r0 + 256, :].rearrange("(t p) e -> p t e",
                                                           p=128),
                        )
                    h32s.append(h32[:, 0, :])
                    h32s.append(h32[:, 1, :])
                return h32s

            def compute(i, h32s):
                x_t = xpool.tile([128, SC, C], F32, tag="x")
                nc.gpsimd.dma_start(
                    x_t[:],
                    x_ap[i * SC * 128:(i + 1) * SC * 128, :].rearrange(
                        "(t p) c -> p t c", p=128
                    ),
                )
                # fp32->bf16 casts with Dv row-sums fused via accum_out,
                # alternating ACT/DVE (explicit TensorReduce has no fast
                # DVE mode, so fusion is the only affordable reduction).
                h16s = []
                for t in range(SC):
                    ci = i * SC + t
                    h16 = h16p.tile([128, E], BF16, tag="h16",
                                    name=f"h16_{i}_{t}")
                    if t % 2 == 0:
                        nc.scalar.activation(
                            h16[:], h32s[t], COPY,
                            accum_out=DvRaw[:, ci:ci + 1],
                        )
                    else:
                        nc.vector.tensor_scalar(
                            h16[:], h32s[t], 0.0, None, ADD, ADD,
                            accum_out=DvRaw[:, ci:ci + 1],
                        )
                    h16s.append(h16)
                rec = spool.tile([128, SC], F32, tag="rec")
                nc.vector.tensor_scalar_add(
                    rec[:], DvRaw[:, i * SC:(i + 1) * SC], EPS
                )
                nc.vector.reciprocal(rec[:], rec[:])
                nc.scalar.sqrt(Isd[:, i * SC:(i + 1) * SC], rec[:])

                xs16 = xpool.tile([128, SC, C], BF16, tag="xs")
                for t in range(SC):
                    ci = i * SC + t
                    if t % 2 == 0:
                        nc.scalar.mul(
                            xs16[:, t, :], x_t[:, t, :], Isd[:, ci:ci + 1]
                        )
                    else:
                        nc.vector.tensor_scalar_mul(
                            xs16[:, t, :], x_t[:, t, :], Isd[:, ci:ci + 1]
                        )

                # out2T matmuls first: a single short gate (xs chain), so
                # the long transpose/De tail below runs ungated -- by then
                # every cast has landed and PE free-runs to the superchunk
                # end, freeing h16 buffers at a steady pace for the DMA.
                for t in range(SC):
                    for s in range(4):
                        nc.tensor.matmul(
                            out2T_ps[:, s * 512:(s + 1) * 512],
                            xs16[:, t, :],
                            h16s[t][:, s * 512:(s + 1) * 512],
                            start=(i == 0 and t == 0),
                            stop=(i == NSUPER - 1 and t == SC - 1),
                        )

                # De partials for this superchunk on the PE: tiny Nf=1
                # ones-matmuls accumulated over the 4 chunks per e-tile.
                # Keeps the staging copies accum_out-free (the DVE/ACT
                # fused-reduce tax was the pass-1 bottleneck).
                deps = psT.tile([128, ETILES], F32, tag="stg",
                                name=f"deps_{i}")
                for j in range(ETILES):
                    stg = psT.tile([128, SC * 128], BF16, tag="stg")
                    for t in range(SC):
                        nc.tensor.transpose(
                            stg[:, t * 128:(t + 1) * 128],
                            h16s[t][:, j * 128:(j + 1) * 128],
                            ident16[:],
                        )
                    for t in range(SC):
                        nc.tensor.matmul(
                            deps[:, j:j + 1],
                            h16s[t][:, j * 128:(j + 1) * 128],
                            ones_n[:], start=(t == 0), stop=(t == SC - 1),
                        )
                    dest = HT3[:, j, i * SC * 128:(i + 1) * SC * 128]
                    # 7 copies on ACT, 9 on DVE (ACT also carries 2 casts
                    # and is the tighter engine)
                    if j % 2 == 0 and j != 14:
                        nc.scalar.copy(dest, stg[:])
                    else:
                        nc.vector.tensor_copy(dest, stg[:])
                nc.scalar.copy(DeP3[:, :, i:i + 1], deps[:].rearrange(
                    "p (j o) -> p j o", o=1))

            h32s_cur = load(0)
            h32s_next = load(1)
            compute(0, h32s_cur)
            h32s_cur = h32s_next

            # W / b prep AFTER superchunk 0's compute: nothing on the H
            # critical path (first casts, first transposes) then queues
            # behind the W DMA -> wt16 cast chain at startup.
            wt32 = spool.tile([128, 128], F32, tag="wt32")
            nc.gpsimd.dma_start(wt32[:], W_d.ap())
            wt16 = const.tile([128, 128], BF16)          # W^T: [c_in, c_out]
            nc.vector.tensor_copy(wt16[:], wt32[:])

            b_sb = const.tile([1, 128], F32)
            nc.gpsimd.dma_start(b_sb[:], b_d.ap())
            ones1 = const.tile([1, 128], F32)
            nc.vector.memset(ones1[:], 1.0)
            bb_ps = psT.tile([128, 128], F32, tag="stg")
            nc.tensor.matmul(bb_ps[:], ones1[:], b_sb[:], start=True, stop=True)
            b_bcast = const.tile([128, 128], F32)        # b replicated per row
            nc.scalar.copy(b_bcast[:], bb_ps[:])

            for i in range(1, NSUPER):
                h32s_next = load(i + 1) if i + 1 < NSUPER else None
                compute(i, h32s_cur)
                h32s_cur = h32s_next

            # --- interlude: De totals, copy out2 out of PSUM ---------------
            nc.vector.reduce_sum(RecDe[:], DeP3[:, :, :], axis=X)
            nc.vector.tensor_scalar_add(RecDe[:], RecDe[:], EPS)
            nc.vector.reciprocal(RecDe[:], RecDe[:])

            # out2T lands (bf16) in out3's buffer; each e-tile is then
            # transposed out and the scaled result overwrites it in place.
            nc.scalar.copy(out3[:, 0:1024], out2T_ps[:, 0:1024])
            nc.vector.tensor_copy(out3[:, 1024:2048], out2T_ps[:, 1024:2048])

            psA_cm.__exit__(None, None, None)

            # --- pass 2: 4 groups of column blocks (3+2+2+1) ---------------
            # Group 0's matmul stream is interleaved with the out3 build
            # (transpose + 1/De scale per e-tile); each group's epilogue is
            # emitted after the NEXT group's stream so PE stays dense, and
            # the last group is a single block to minimize the exposed tail.
            GROUPS = [[0, 1], [2, 3], [4, 5], [6], [7]]
            psB_cm = tc.tile_pool(name="psB", bufs=4, space="PSUM")
            psB = psB_cm.__enter__()

            o4 = {}
            for grp in GROUPS:
                for blk in grp:
                    o4[blk] = psB.tile([128, 512], F32, tag="o4",
                                       name=f"o4_{blk}")

            def jstream(g):
                for j in range(ETILES):
                    for blk in GROUPS[g]:
                        nc.tensor.matmul(
                            o4[blk][:],
                            out3[:, j * 128:(j + 1) * 128],
                            HT[:, j * N + blk * 512:j * N + (blk + 1) * 512],
                            start=(j == 0), stop=(j == ETILES - 1),
                        )

            # group 0 + out3 build, interleaved per e-tile
            for j in range(ETILES):
                t2 = psT.tile([128, 128], BF16, tag="stg")
                nc.tensor.transpose(
                    t2[:], out3[:, j * 128:(j + 1) * 128], ident16[:]
                )
                if j % 2 == 0:
                    nc.scalar.mul(
                        out3[:, j * 128:(j + 1) * 128], t2[:],
                        RecDe[:, j:j + 1]
                    )
                else:
                    nc.vector.tensor_scalar_mul(
                        out3[:, j * 128:(j + 1) * 128], t2[:],
                        RecDe[:, j:j + 1]
                    )
                for blk in GROUPS[0]:
                    nc.tensor.matmul(
                        o4[blk][:],
                        out3[:, j * 128:(j + 1) * 128],
                        HT[:, j * N + blk * 512:j * N + (blk + 1) * 512],
                        start=(j == 0), stop=(j == ETILES - 1),
                    )

            def epilogue(g, halves=False):
                for blk in GROUPS[g]:
                    # all o4sb copies on ACT: keeps them out of the DVE
                    # queue so lp matmuls never wait behind queued stt's
                    o4sb = opool.tile([128, 512], BF16, tag="o4sb")
                    nc.scalar.copy(o4sb[:], o4[blk][:])
                    # tail blocks store in 256-row halves so the last DMA
                    # starts as soon as its half of the results is ready
                    nh = 2 if halves else 1
                    for h in range(nh):
                        obig = opool.tile([128, 4 // nh, C], F32,
                                          tag="obig", name=f"obig{blk}_{h}")
                        for u in range(4 // nh):
                            t = h * (4 // nh) + u
                            idx = blk * 4 + t
                            lp = psT.tile([128, 128], F32, tag="stg",
                                          name=f"lp_{blk}_{t}")
                            nc.tensor.matmul(
                                lp[:], o4sb[:, t * 128:(t + 1) * 128],
                                wt16[:], start=True, stop=True,
                            )
                            nc.vector.scalar_tensor_tensor(
                                obig[:, u, :], lp[:], Isd[:, idx:idx + 1],
                                b_bcast[:],
                                mybir.AluOpType.mult, mybir.AluOpType.add,
                            )
                        r0 = blk * 512 + h * (512 // nh)
                        nc.gpsimd.dma_start(
                            out_ap[r0:r0 + 512 // nh, :].rearrange(
                                "(t p) c -> p t c", p=128
                            ),
                            obig[:],
                        )

            jstream(1)
            epilogue(0)
            jstream(2)
            epilogue(1)
            jstream(3)
            epilogue(2)
            jstream(4)
            epilogue(3, halves=True)
            epilogue(4, halves=True)

            psB_cm.__exit__(None, None, None)
            psT_cm.__exit__(None, None, None)

    nc.compile()
    return nc


def _get_nc():
    if "nc" not in _CACHE:
        _CACHE["nc"] = _build_nc()
    return _CACHE["nc"]


def kernel(x, H, W, b):
    from concourse.bass_utils import run_bass_kernel_spmd

    nc = _get_nc()
    x = np.ascontiguousarray(x, dtype=np.float32)
    H = np.ascontiguousarray(H, dtype=np.float32)
    W = np.ascontiguousarray(W, dtype=np.float32)
    b2 = np.ascontiguousarray(b, dtype=np.float32).reshape(1, C)
    Wt = np.ascontiguousarray(W.T)
    in_maps = [
        {"x": x[c], "H": H[c], "Wt": Wt, "b": b2} for c in range(B)
    ]
    res = run_bass_kernel_spmd(nc, in_maps, core_ids=list(range(B)))
    return np.stack([res.results[c]["out"] for c in range(B)], axis=0)



# revision 21
# speedup vs baseline: 1.1977x; 1.0162x over previous
"""Trainium2 Bass kernel for a batched HGNN layer.

Per batch b (N=4096 nodes, E=2048 hyperedges, C=128 channels):
    De = sum_n H[n,e] + eps                 (hyperedge degrees)
    Dv = sum_e H[n,e] + eps                 (node degrees)
    s  = 1/sqrt(Dv)
    out = ((H @ ((H^T @ (x * s)) / De)) * s) @ W^T + b

Sharding: batch dim B=8, one batch per NeuronCore (data parallel, no
cross-core communication). Inside a core:

  pass 1 (streams H once from HBM in 8 superchunks of 512 rows,
          software-pipelined, fp32->bf16 cast done by the SWDGE DMA
          itself so ACT/DVE only do copies/reductions):
    - Dv row-sums via DVE reduce on the bf16 tile
    - out2T[c,e] = (x*s)^T @ H accumulated in PSUM (PE, bf16)
    - H^T built with PE transposes, staged via PSUM, copied to a
      16 MB bf16 SBUF cache with plain ACT/DVE copies (no accum_out)
    - De col-sums via grouped DVE reduces over the fresh H^T slices
  interlude (fused into pass 2's first block-pair stream):
    - out3[e,c] = transpose(out2T) * (1/De)
  pass 2 (H^T streamed from SBUF, no HBM traffic; 4 groups of 2
          column blocks so each group's epilogue overlaps the next
          group's matmul stream):
    - out4T[c,n] = out3^T @ H^T  (PE, bf16)
    - out[n,co] = (out4T_tile^T @ W^T)*s + b  (PE bf16, DVE epilogue)

HBM traffic per core = 32 MB (H) + 2 MB (x) + 2 MB (out) ~= 36 MB,
i.e. the memory roofline for this problem.
"""
import os
import sys

import numpy as np

for _p in ("/opt/trn_rl_repo", "/root/.axon_site/_ro/trn_rl_repo"):
    if os.path.isdir(_p) and _p not in sys.path:
        sys.path.append(_p)

B, N, E, C = 8, 4096, 2048, 128
SC = 4                      # subchunks (128 rows) per superchunk
KHT = 130                   # H^T cache block: 128 cols + De partial + pad
                            # (130 keeps each PSUM transpose dest 4B-aligned)
NSUPER = N // (128 * SC)    # 8 superchunks in pass 1
NCHUNKS = N // 128          # 32 row chunks
ETILES = E // 128           # 16 hyperedge tiles
EPS = 1e-6

_CACHE = {}


def _build_nc():
    from contextlib import ExitStack

    import concourse.tile as tile
    from concourse import bacc, mybir

    F32 = mybir.dt.float32
    BF16 = mybir.dt.bfloat16
    X = mybir.AxisListType.X
    XY = mybir.AxisListType.XY
    COPY = mybir.ActivationFunctionType.Copy
    ADD = mybir.AluOpType.add

    nc = bacc.Bacc("TRN2", target_bir_lowering=False, debug=False)

    H_d = nc.dram_tensor("H", [N, E], F32, kind="ExternalInput")
    x_d = nc.dram_tensor("x", [N, C], F32, kind="ExternalInput")
    W_d = nc.dram_tensor("Wt", [C, C], F32, kind="ExternalInput")
    b_d = nc.dram_tensor("b", [1, C], F32, kind="ExternalInput")
    out_d = nc.dram_tensor("out", [N, C], F32, kind="ExternalOutput")

    H_ap, x_ap, out_ap = H_d.ap(), x_d.ap(), out_d.ap()

    with tile.TileContext(nc) as tc:
        with ExitStack() as ctx:
            const = ctx.enter_context(tc.tile_pool(name="const", bufs=1))
            h32p = ctx.enter_context(tc.tile_pool(name="h32", bufs=2))
            h16p = ctx.enter_context(tc.tile_pool(name="h16", bufs=6))
            xpool = ctx.enter_context(tc.tile_pool(name="xp", bufs=2))
            spool = ctx.enter_context(tc.tile_pool(name="sp", bufs=2))
            opool = ctx.enter_context(tc.tile_pool(name="op", bufs=2))
            psT_cm = tc.tile_pool(name="psT", bufs=4, space="PSUM")
            psT = psT_cm.__enter__()
            psA_cm = tc.tile_pool(name="psA", bufs=1, space="PSUM")
            psA = psA_cm.__enter__()

            # --- constants -------------------------------------------------
            # Extended identity [I | 1 1]: transposing with it makes columns
            # 128/129 of each PE transpose the partition-sum of the input
            # tile, i.e. a free De (hyperedge-degree) partial -- no
            # accum_out needed on the staging copies.
            ident16 = const.tile([128, 128], BF16)
            nc.vector.memset(ident16[:], 1.0)
            nc.gpsimd.affine_select(
                ident16[:], ident16[:], pattern=[[-1, 128]], base=0,
                channel_multiplier=1, compare_op=mybir.AluOpType.is_equal,
                fill=0.0,
            )
            ones_n = const.tile([128, 1], BF16)
            nc.vector.memset(ones_n[:], 1.0)

            # --- persistent state ------------------------------------------
            HT = const.tile([128, ETILES * N], BF16)     # H^T cache, 128 KB/part
            out3 = const.tile([128, ETILES * 128], BF16)  # (H^T xs)/De, [e, c]
            Isd = const.tile([128, NCHUNKS], F32)        # 1/sqrt(Dv)
            DvRaw = const.tile([128, NCHUNKS], F32)
            DeP2 = const.tile([128, ETILES * NSUPER], F32)  # De partials
            RecDe = const.tile([128, ETILES], F32)

            out2T_ps = psA.tile([128, E], F32)           # 4 PSUM banks

            HT3 = HT[:].rearrange("p (j n) -> p j n", j=ETILES)
            DeP3 = DeP2[:].rearrange("p (j i) -> p j i", j=ETILES)

            # --- pass 1 (software pipelined) -------------------------------
            def load(i):
                """DMA superchunk i (fp32) on the sync HWDGE ring.

                All H chunks go on nc.sync: the sync engine is otherwise
                idle, so triggers issue immediately.  (nc.scalar triggers
                sit in the busy ACT queue and stall the stream.)  2 MiB per
                call (2 row-chunks) for better DMA efficiency and fewer
                completion gaps.
                """
                h32s = []
                for h in range(SC // 2):
                    h32 = h32p.tile([128, 2, E], F32, tag="h32")
                    r0 = (i * SC + 2 * h) * 128
                    nc.sync.dma_start(
                        h32[:],
                        H_ap[r0:r0 + 256, :].rearrange("(t p) e -> p t e",
                                                       p=128),
                    )
                    h32s.append(h32[:, 0, :])
                    h32s.append(h32[:, 1, :])
                return h32s

            DSPLIT = ETILES // 2    # deps for j >= DSPLIT defer past the
                                    # superchunk boundary as PE filler

            def emit_casts(i, h32s):
                """x prefetch + fp32->bf16 casts (with fused Dv accum) for
                superchunk i. Emitted one superchunk EARLY so the
                cast->Dv->rsqrt->xs chain is resolved (except the last
                cast, ~2.3us) by the time superchunk i's matmuls issue.
                """
                x_t = xpool.tile([128, SC, C], F32, tag="x", name=f"x_{i}")
                nc.gpsimd.dma_start(
                    x_t[:],
                    x_ap[i * SC * 128:(i + 1) * SC * 128, :].rearrange(
                        "(t p) c -> p t c", p=128
                    ),
                )
                h16s = [emit_cast_one(i, t, h32s) for t in range(SC)]
                return x_t, h16s

            def emit_cast_one(i, t, h32s):
                ci = i * SC + t
                h16 = h16p.tile([128, E], BF16, tag="h16",
                                name=f"h16_{i}_{t}")
                if t % 2 == 0:
                    nc.scalar.activation(
                        h16[:], h32s[t], COPY,
                        accum_out=DvRaw[:, ci:ci + 1],
                    )
                else:
                    nc.vector.tensor_scalar(
                        h16[:], h32s[t], 0.0, None, ADD, ADD,
                        accum_out=DvRaw[:, ci:ci + 1],
                    )
                return h16

            def emit_xs(i, x_t):
                rec = spool.tile([128, SC], F32, tag="rec")
                nc.vector.tensor_scalar_add(
                    rec[:], DvRaw[:, i * SC:(i + 1) * SC], EPS
                )
                nc.vector.reciprocal(rec[:], rec[:])
                nc.scalar.sqrt(Isd[:, i * SC:(i + 1) * SC], rec[:])
                xs16 = xpool.tile([128, SC, C], BF16, tag="xs")
                for t in range(SC):
                    ci = i * SC + t
                    if t % 2 == 0:
                        nc.scalar.mul(
                            xs16[:, t, :], x_t[:, t, :], Isd[:, ci:ci + 1]
                        )
                    else:
                        nc.vector.tensor_scalar_mul(
                            xs16[:, t, :], x_t[:, t, :], Isd[:, ci:ci + 1]
                        )
                return xs16

            def emit_mms(i, xs16, h16s):
                for t in range(SC):
                    for s in range(4):
                        nc.tensor.matmul(
                            out2T_ps[:, s * 512:(s + 1) * 512],
                            xs16[:, t, :],
                            h16s[t][:, s * 512:(s + 1) * 512],
                            start=(i == 0 and t == 0),
                            stop=(i == NSUPER - 1 and t == SC - 1),
                        )

            def emit_jloop(i, h16s, deps, j0=0, j1=ETILES):
                """Transposes + staging copies, and deps (De partial)
                ones-matmuls for the first DSPLIT e-tiles only; the rest
                defer to the next superchunk boundary as PE filler."""
                for j in range(j0, j1):
                    stg = psT.tile([128, SC * 128], BF16, tag="stg")
                    for t in range(SC):
                        nc.tensor.transpose(
                            stg[:, t * 128:(t + 1) * 128],
                            h16s[t][:, j * 128:(j + 1) * 128],
                            ident16[:],
                        )
                    if j < DSPLIT:
                        for t in range(SC):
                            nc.tensor.matmul(
                                deps[:, j:j + 1],
                                h16s[t][:, j * 128:(j + 1) * 128],
                                ones_n[:], start=(t == 0),
                                stop=(t == SC - 1),
                            )
                    dest = HT3[:, j, i * SC * 128:(i + 1) * SC * 128]
                    # 7 copies on ACT, 9 on DVE (ACT also carries 2 casts
                    # and is the tighter engine)
                    if j % 2 == 0 and j != 14:
                        nc.scalar.copy(dest, stg[:])
                    else:
                        nc.vector.tensor_copy(dest, stg[:])

            def emit_deps_tail(i, h16s, deps):
                """Deferred De ones-matmuls: fill the PE while the next
                superchunk's last cast -> rsqrt -> xs chain resolves."""
                for j in range(DSPLIT, ETILES):
                    for t in range(SC):
                        nc.tensor.matmul(
                            deps[:, j:j + 1],
                            h16s[t][:, j * 128:(j + 1) * 128],
                            ones_n[:], start=(t == 0), stop=(t == SC - 1),
                        )
                nc.scalar.copy(DeP3[:, :, i:i + 1], deps[:].rearrange(
                    "p (j o) -> p j o", o=1))

            # --- prologue ---------------------------------------------------
            h32s_by_sc = {0: load(0)}
            xh_cur = emit_casts(0, h32s_by_sc[0])
            h32s_by_sc[1] = load(1)

            # W / b prep: gpsimd DMAs + PE/DVE are idle at startup, and
            # the H stream on the sync ring is not delayed by these.
            wt32 = spool.tile([128, 128], F32, tag="wt32")
            nc.gpsimd.dma_start(wt32[:], W_d.ap())
            wt16 = const.tile([128, 128], BF16)          # W^T: [c_in, c_out]
            nc.vector.tensor_copy(wt16[:], wt32[:])

            b_sb = const.tile([1, 128], F32)
            nc.gpsimd.dma_start(b_sb[:], b_d.ap())
            ones1 = const.tile([1, 128], F32)
            nc.vector.memset(ones1[:], 1.0)
            bb_ps = psT.tile([128, 128], F32, tag="stg")
            nc.tensor.matmul(bb_ps[:], ones1[:], b_sb[:], start=True, stop=True)
            b_bcast = const.tile([128, 128], F32)        # b replicated per row
            nc.scalar.copy(b_bcast[:], bb_ps[:])

            # --- main pipelined loop ---------------------------------------
            # Per iteration: xs chain for i (casts already done), deferred
            # deps of i-1 as PE boundary filler, out2T matmuls, then the
            # j-loop with next superchunk's casts interleaved so each cast
            # sits in its engine queue just ahead of its DMA completing.
            prev = None   # (i-1, h16s, deps) with deferred deps tail
            for i in range(NSUPER):
                x_t, h16s = xh_cur
                xs16 = emit_xs(i, x_t)
                if prev is not None:
                    emit_deps_tail(*prev)
                deps = psT.tile([128, ETILES], F32, tag="stg",
                                name=f"deps_{i}")
                last = i + 1 >= NSUPER
                if i == 0:
                    # superchunk 0: transposes first -- they only need the
                    # casts as they land, while the xs chain needs all 4.
                    emit_jloop(i, h16s, deps)
                    emit_mms(i, xs16, h16s)
                    if not last:
                        xh_cur = emit_casts(i + 1, h32s_by_sc[i + 1])
                    if i + 2 < NSUPER:
                        h32s_by_sc[i + 2] = load(i + 2)
                else:
                    emit_mms(i, xs16, h16s)
                    if not last:
                        x_n = xpool.tile([128, SC, C], F32, tag="x",
                                         name=f"x_{i + 1}")
                        nc.gpsimd.dma_start(
                            x_n[:],
                            x_ap[(i + 1) * SC * 128:
                                 (i + 2) * SC * 128, :].rearrange(
                                "(t p) c -> p t c", p=128
                            ),
                        )
                        h16n = [emit_cast_one(i + 1, 0, h32s_by_sc[i + 1])]
                    if i + 2 < NSUPER:
                        h32s_by_sc[i + 2] = load(i + 2)
                    emit_jloop(i, h16s, deps, 0, DSPLIT)
                    if not last:
                        h16n.append(
                            emit_cast_one(i + 1, 1, h32s_by_sc[i + 1]))
                    emit_jloop(i, h16s, deps, DSPLIT, ETILES)
                    if not last:
                        h16n.append(
                            emit_cast_one(i + 1, 2, h32s_by_sc[i + 1]))
                        h16n.append(
                            emit_cast_one(i + 1, 3, h32s_by_sc[i + 1]))
                        xh_cur = (x_n, h16n)
                prev = (i, h16s, deps)
            emit_deps_tail(*prev)

            # --- interlude: De totals, copy out2 out of PSUM ---------------
            nc.vector.reduce_sum(RecDe[:], DeP3[:, :, :], axis=X)
            nc.vector.tensor_scalar_add(RecDe[:], RecDe[:], EPS)
            nc.vector.reciprocal(RecDe[:], RecDe[:])

            # out2T lands (bf16) in out3's buffer; each e-tile is then
            # transposed out and the scaled result overwrites it in place.
            nc.scalar.copy(out3[:, 0:1024], out2T_ps[:, 0:1024])
            nc.vector.tensor_copy(out3[:, 1024:2048], out2T_ps[:, 1024:2048])

            psA_cm.__exit__(None, None, None)

            # --- pass 2: 4 groups of column blocks (3+2+2+1) ---------------
            # Group 0's matmul stream is interleaved with the out3 build
            # (transpose + 1/De scale per e-tile); each group's epilogue is
            # emitted after the NEXT group's stream so PE stays dense, and
            # the last group is a single block to minimize the exposed tail.
            GROUPS = [[0, 1], [2, 3], [4, 5], [6], [7]]
            psB_cm = tc.tile_pool(name="psB", bufs=4, space="PSUM")
            psB = psB_cm.__enter__()

            o4 = {}
            for grp in GROUPS:
                for blk in grp:
                    o4[blk] = psB.tile([128, 512], F32, tag="o4",
                                       name=f"o4_{blk}")

            def jstream(g):
                for j in range(ETILES):
                    for blk in GROUPS[g]:
                        nc.tensor.matmul(
                            o4[blk][:],
                            out3[:, j * 128:(j + 1) * 128],
                            HT[:, j * N + blk * 512:j * N + (blk + 1) * 512],
                            start=(j == 0), stop=(j == ETILES - 1),
                        )

            # group 0 + out3 build, interleaved per e-tile
            for j in range(ETILES):
                t2 = psT.tile([128, 128], BF16, tag="stg")
                nc.tensor.transpose(
                    t2[:], out3[:, j * 128:(j + 1) * 128], ident16[:]
                )
                if j % 2 == 0:
                    nc.scalar.mul(
                        out3[:, j * 128:(j + 1) * 128], t2[:],
                        RecDe[:, j:j + 1]
                    )
                else:
                    nc.vector.tensor_scalar_mul(
                        out3[:, j * 128:(j + 1) * 128], t2[:],
                        RecDe[:, j:j + 1]
                    )
                for blk in GROUPS[0]:
                    nc.tensor.matmul(
                        o4[blk][:],
                        out3[:, j * 128:(j + 1) * 128],
                        HT[:, j * N + blk * 512:j * N + (blk + 1) * 512],
                        start=(j == 0), stop=(j == ETILES - 1),
                    )

            def epilogue(g):
                for blk in GROUPS[g]:
                    # all o4sb copies on ACT: keeps them out of the DVE
                    # queue so lp matmuls never wait behind queued stt's
                    o4sb = opool.tile([128, 512], BF16, tag="o4sb")
                    nc.scalar.copy(o4sb[:], o4[blk][:])
                    obig = opool.tile([128, 4, C], F32, tag="obig",
                                      name=f"obig{blk}")
                    for t in range(4):
                        idx = blk * 4 + t
                        lp = psT.tile([128, 128], F32, tag="stg")
                        nc.tensor.matmul(
                            lp[:], o4sb[:, t * 128:(t + 1) * 128], wt16[:],
                            start=True, stop=True,
                        )
                        nc.vector.scalar_tensor_tensor(
                            obig[:, t, :], lp[:], Isd[:, idx:idx + 1],
                            b_bcast[:],
                            mybir.AluOpType.mult, mybir.AluOpType.add,
                        )
                    nc.gpsimd.dma_start(
                        out_ap[blk * 512:(blk + 1) * 512, :].rearrange(
                            "(t p) c -> p t c", p=128
                        ),
                        obig[:],
                    )

            jstream(1)
            epilogue(0)
            jstream(2)
            epilogue(1)
            jstream(3)
            epilogue(2)
            jstream(4)
            epilogue(3)
            epilogue(4)

            psB_cm.__exit__(None, None, None)
            psT_cm.__exit__(None, None, None)

    nc.compile()
    return nc


def _get_nc():
    if "nc" not in _CACHE:
        _CACHE["nc"] = _build_nc()
    return _CACHE["nc"]


def kernel(x, H, W, b):
    from concourse.bass_utils import run_bass_kernel_spmd

    nc = _get_nc()
    x = np.ascontiguousarray(x, dtype=np.float32)
    H = np.ascontiguousarray(H, dtype=np.float32)
    W = np.ascontiguousarray(W, dtype=np.float32)
    b2 = np.ascontiguousarray(b, dtype=np.float32).reshape(1, C)
    Wt = np.ascontiguousarray(W.T)
    in_maps = [
        {"x": x[c], "H": H[c], "Wt": Wt, "b": b2} for c in range(B)
    ]
    res = run_bass_kernel_spmd(nc, in_maps, core_ids=list(range(B)))
    return np.stack([res.results[c]["out"] for c in range(B)], axis=0)

